# revision 1
# baseline (speedup 1.0000x reference)
"""Trainium2 Bass kernel for nn_ChaosTransformer_22333829939822.

Key mathematical reduction (verified against the reference):
the torch-style ``view(B, H, L, E//H)`` on a [B, L, E] tensor is a raw
row-major reshape, which makes head h attend only within the 256-position
block [h*256, (h+1)*256).  The output ``dec[:, -96:, 0]`` therefore depends
only on the last 256 positions of each batch.  Each core runs one batch's
[256, 256] residual-stream transformer; attention operates on the
[2048, 32] head-view of the 256x256 block.

Sharding: data-parallel over batch B across 4 of the 8 cores (one batch
per core, fully independent, no collectives).

Layouts on device:
- residual stream X kept position-major ([pos, ch], for LayerNorm) and
  channel-major XT ([ch, pos], matmul operand), fp32 bitcast to float32r
  for full-rate PE matmuls at N>=256.
- attention in bf16.  Scores are built KEY-major: ST_c[s, q] so that the
  exp'd tiles feed A@V directly as the moving operand with keys on the
  contraction (partition) axis.  Key order is re-enumerated as
  s = (c', pc, p') which makes the head-view V slices plain slices of
  position-major V.  Row sums for the softmax denominator come from
  ones-column matmuls; normalization happens once at the end (scores are
  provably tiny here: |SCALE*decay*S| < ~3, so exp needs no max shift).
- the query c-block axis lives on SBUF partitions; 4x row-packed K=32
  matmuls need the Q tile at 4 rotations of its 32-partition blocks,
  produced by permutation matmuls (host supplies the 0/1 matrices).
"""

import sys
import numpy as np

sys.path.insert(0, "/opt/trn_rl_repo")

import concourse.bass as bass
import concourse.tile as tile
from concourse import mybir
from concourse.masks import make_identity
from concourse.tile_rust import add_dep_helper

F32 = mybir.dt.float32
F32R = mybir.dt.float32r
BF16 = mybir.dt.bfloat16
# matmul operand dtype for projections/residual-stream operands.
# "bf16" = fast (1 cyc/row); "f32" = exact (4 cyc/row).
MM_DTYPE = "mixed"
STAGE = 99   # debug: truncate kernel after stage N
ADD = mybir.AluOpType.add
SUB = mybir.AluOpType.subtract
MULT = mybir.AluOpType.mult
MAX = mybir.AluOpType.max
AF = mybir.ActivationFunctionType

B, L, D, E, DFF, LYR, PRED = 4, 2048, 7, 256, 1024, 2, 96
FACTOR = 5.0
SCALE = 1.0 / float(np.sqrt(FACTOR))
EPS = 1e-5
P0 = L - 256          # 1792: start of the last 256-position block
QLO2 = 128            # layer-2 computes query positions [128, 256)
                      # (output needs [160, 256); 128 keeps tiles base-0 aligned)
NPOS = 256


def chaos_kernel(tc, outs, ins):
    import contextlib

    nc = tc.nc
    with contextlib.ExitStack() as ctx:
        _chaos_body(tc, nc, ctx, outs, ins)


def _chaos_body(tc, nc, ctx, outs, ins):
    WDT = F32 if MM_DTYPE == "f32" else BF16
    const = ctx.enter_context(tc.tile_pool(name="const", bufs=1))
    work = ctx.enter_context(tc.tile_pool(name="work", bufs=3))
    atp = ctx.enter_context(tc.tile_pool(name="atp", bufs=24))
    psw = ctx.enter_context(tc.tile_pool(name="psw", bufs=3, space="PSUM"))
    psacc = ctx.enter_context(tc.tile_pool(name="psacc", bufs=1, space="PSUM"))
    drp = ctx.enter_context(tc.tile_pool(name="drp", bufs=2, space="DRAM"))

    dma = nc.sync.dma_start

    def seed_bias(ps_ap, brow_ap, m, n):
        """PSUM <- bias row broadcast over m partitions (K=1 matmul)."""
        ones = ones_row if brow_ap.dtype == F32 else ones_row_w
        nc.tensor.matmul(
            ps_ap, ones[0:1, :m], brow_ap,
            start=True, stop=False,
        )

    def layernorm(x_ap, rows, g_b, b_b, out_ap):
        st = work.tile([128, 6], F32, tag="bn_st")
        nc.vector.bn_stats(st[:rows], x_ap)
        mv = work.tile([128, 2], F32, tag="bn_mv")
        nc.vector.bn_aggr(mv[:rows], st[:rows])
        sd = work.tile([128, 1], F32, tag="bn_sd")
        nc.scalar.activation(sd[:rows], mv[:rows, 1:2], AF.Sqrt,
                             bias=eps_t[:rows])
        nc.vector.reciprocal(sd[:rows], sd[:rows])
        t = work.tile([128, NPOS], F32, tag="ln_t")
        nc.vector.tensor_scalar(t[:rows], x_ap, mv[:rows, 0:1], sd[:rows],
                                SUB, MULT)
        nc.vector.tensor_mul(t[:rows], t[:rows], g_b[:rows])
        nc.vector.tensor_add(out_ap, t[:rows], b_b[:rows])

    # ---------------- constant loads ----------------
    xT_sb = const.tile([D, NPOS], F32, tag="xT")
    dma(out=xT_sb[:], in_=ins["xT"][:])
    Wemb_sb = const.tile([D, E], F32, tag="Wemb")
    dma(out=Wemb_sb[:], in_=ins["Wemb"][:])

    Wq_t, Wk_t, Wv_t, Wo_t, W1_t, W2_t = {}, {}, {}, {}, {}, {}
    for l in range(LYR):
        for k in range(2):
            for nm, store in (("Wq", Wq_t), ("Wk", Wk_t), ("Wv", Wv_t)):
                tl = const.tile([128, E], WDT, tag=f"{nm}{l}{k}")
                dma(out=tl[:], in_=ins[nm][l, k * 128:(k + 1) * 128, :])
                store[(l, k)] = tl
            tl = const.tile([128, DFF], WDT, tag=f"W1{l}{k}")
            dma(out=tl[:], in_=ins["W1"][l, k * 128:(k + 1) * 128, :])
            W1_t[(l, k)] = tl
        for h in range(2):
            tl = const.tile([128, E], WDT, tag=f"Wo{l}{h}")
            dma(out=tl[:], in_=ins["Wo"][l, h * 128:(h + 1) * 128, :])
            Wo_t[(l, h)] = tl
        for dk in range(8):
            tl = const.tile([128, E], BF16, tag=f"W2{l}{dk}")
            dma(out=tl[:], in_=ins["W2bf"][l, dk * 128:(dk + 1) * 128, :])
            W2_t[(l, dk)] = tl

    Wp_sb = const.tile([128, 2], F32, tag="Wp")
    dma(out=Wp_sb[:], in_=ins["Wp2"][:])
    bprow = const.tile([1, 1], F32, tag="bproj")
    dma(out=bprow[:], in_=ins["bproj"][:])

    # per-partition (channel-major) biases: [128, nchunks]
    bq_t, bk_t, b1_t = {}, {}, {}
    for l in range(LYR):
        for nm, store, w in (("bq", bq_t, 2), ("bk", bk_t, 2), ("b1", b1_t, 8)):
            t = const.tile([128, w], F32, tag=f"{nm}{l}")
            dma(out=t[:], in_=ins[nm][l].rearrange("(k p) -> p k", p=128))
            store[l] = t
    bemb_pp = const.tile([128, 2], F32, tag="bembpp")
    dma(out=bemb_pp[:], in_=ins["bemb"].rearrange("(k p) -> p k", p=128))

    # bias rows for PSUM seeding (position-major outputs)
    brows = {}
    for nm in ("bv", "bo", "b2"):
        for l in range(LYR):
            t = const.tile([1, E], WDT, tag=f"{nm}{l}r")
            dma(out=t[:], in_=ins[nm][l:l + 1, :])
            brows[(nm, l)] = t
    bemb_r = const.tile([1, E], F32, tag="bembr")
    dma(out=bemb_r[:], in_=ins["bemb"].rearrange("(o e) -> o e", o=1))

    # LN gain/bias broadcast tiles [128, 256]
    ln_b = {}
    for nm in ("ln1g", "ln1b", "ln2g", "ln2b"):
        for l in range(LYR):
            t = const.tile([128, E], F32, tag=f"{nm}{l}")
            dma(out=t[:], in_=ins[nm][l].partition_broadcast(128))
            ln_b[(nm, l)] = t
    for nm in ("lnfg", "lnfb"):
        t = const.tile([128, E], F32, tag=nm)
        dma(out=t[:], in_=ins[nm].partition_broadcast(128))
        ln_b[nm] = t

    Prot_t = {}
    for r in range(3):
        t = const.tile([128, 128], BF16, tag=f"Prot{r}")
        dma(out=t[:], in_=ins["Prot"][r])
        Prot_t[r] = t

    ident = const.tile([128, 128], F32, tag="ident")
    make_identity(nc, ident[:])
    ones_col = const.tile([128, 1], BF16, tag="ones_col")
    nc.vector.memset(ones_col[:], 1.0)
    ones_row = const.tile([1, 128], F32, tag="ones_row")
    nc.vector.memset(ones_row[:], 1.0)
    ones_row_w = const.tile([1, 128], WDT, tag="ones_row_w")
    nc.vector.memset(ones_row_w[:], 1.0)
    eps_t = const.tile([128, 1], F32, tag="eps")
    nc.vector.memset(eps_t[:], EPS)

    # ---------------- decay tiles ----------------
    td_sb = const.tile([1, L], F32, tag="td")
    dma(out=td_sb[:], in_=ins["td"][:])
    decay = const.tile([1, L], F32, tag="decay")
    nc.scalar.activation(decay[:], td_sb[:], AF.Exp, scale=-1.0 / FACTOR)
    nc.vector.tensor_scalar_mul(decay[:], decay[:], SCALE)
    # D[h][32j+d, q] = SCALE * exp(-td[q*8 + (4h+j)]/F)
    # (bounce through DRAM: stride-0 partition broadcast needs a DRAM source)
    decay_dr = drp.tile([1, L], F32, tag="decay_dr")
    dma(out=decay_dr[:], in_=decay[:])
    decay_v = decay_dr[:].rearrange("o (q c) -> o c q", c=8)  # [1, 8, 256]
    D_t = {}
    for h in range(2):
        t = const.tile([128, NPOS], F32, tag=f"D{h}")
        for j in range(4):
            src = decay_v[0, 4 * h + j, :].partition_broadcast(32)
            dma(out=t[32 * j:32 * (j + 1), :], in_=src)
        D_t[h] = t

    # ---------------- embedding ----------------
    X_t, XT_t = {}, {}
    for p in range(2):  # position-major X
        ps = psw.tile([128, 1024], F32, tag="qk")
        seed_bias(ps[:, :E], bemb_r[0:1, :], 128, E)
        nc.tensor.matmul(ps[:, :E],
                         xT_sb[:, p * 128:(p + 1) * 128],
                         Wemb_sb[:], start=False, stop=True)
        t = const.tile([128, NPOS], F32, tag=f"X{p}")
        nc.vector.tensor_copy(t[:], ps[:, :E])
        X_t[p] = t
    for k in range(2):  # channel-major XT
        ps = psw.tile([128, 1024], F32, tag="qk")
        nc.tensor.matmul(ps[:, :NPOS],
                         Wemb_sb[:, k * 128:(k + 1) * 128],
                         xT_sb[:], start=True, stop=True)
        t = const.tile([128, NPOS], WDT, tag=f"XT{k}")
        nc.vector.tensor_scalar_add(t[:], ps[:, :NPOS], bemb_pp[:, k:k + 1])
        XT_t[k] = t

    def _stub_out():
        ot = work.tile([128, 1], F32, tag="outsb")
        nc.vector.memset(ot[:], 0.0)
        nc.sync.dma_start(out=outs["out"][:], in_=ot[128 - PRED:, :])

    if STAGE < 1:
        _stub_out()
        return

    # ---------------- transformer layers ----------------
    for l in range(LYR):
        qlo, qhi = (0, NPOS) if l == 0 else (QLO2, NPOS)
        qw = qhi - qlo
        pos_chunks = ([(0, 0, 128), (1, 0, 128)] if l == 0
                      else [(1, 0, 128)])
        # (X-tile index, row offset within tile, nrows) for output positions

        # ---- K projection -> KT channel-major bf16 [128, 256] x2
        KT = {}
        for Jt in range(2):
            ps = psw.tile([128, 1024], F32, tag="qk")
            for k in range(2):
                nc.tensor.matmul(
                    ps[:, :NPOS],
                    Wk_t[(l, k)][:, Jt * 128:(Jt + 1) * 128],
                    XT_t[k][:],
                    start=(k == 0), stop=(k == 1))
            t = work.tile([128, NPOS], BF16, tag=f"KT{Jt}")
            nc.vector.tensor_scalar_add(t[:], ps[:, :NPOS], bk_t[l][:, Jt:Jt + 1])
            KT[Jt] = t

        # ---- V projection -> V position-major bf16 [128, 256] x2
        V = {}
        for pc in range(2):
            ps = psw.tile([128, 1024], F32, tag="qk")
            seed_bias(ps[:, :E], brows[("bv", l)][0:1, :], 128, E)
            for k in range(2):
                nc.tensor.matmul(
                    ps[:, :E],
                    XT_t[k][:, pc * 128:(pc + 1) * 128],
                    Wv_t[(l, k)][:],
                    start=False, stop=(k == 1))
            t = work.tile([128, E], BF16, tag=f"V{pc}")
            nc.vector.tensor_copy(t[:], ps[:, :E])
            V[pc] = t

        # ---- Q projection -> Qs (decay-scaled) bf16, rotations r=0..3
        Qs = {}
        for h in range(2):
            ps = psw.tile([128, 1024], F32, tag="qk")
            for k in range(2):
                nc.tensor.matmul(
                    ps[:, :qw],
                    Wq_t[(l, k)][:, h * 128:(h + 1) * 128],
                    XT_t[k][:, qlo:qhi],
                    start=(k == 0), stop=(k == 1))
            tf = work.tile([128, NPOS], F32, tag="qtmp")
            nc.vector.tensor_scalar_add(tf[:, :qw], ps[:, :qw],
                                        bq_t[l][:, h:h + 1])
            t = work.tile([128, NPOS], BF16, tag=f"Qs0{h}")
            nc.vector.tensor_mul(t[:, :qw], tf[:, :qw], D_t[h][:, qlo:qhi])
            Qs[(0, h)] = t
        for r in range(1, 4):
            for h in range(2):
                ps = psw.tile([128, 1024], F32, tag="qk")
                nc.tensor.matmul(ps[:, :qw], Prot_t[r - 1][:],
                                 Qs[(0, h)][:, :qw], start=True, stop=True)
                t = work.tile([128, NPOS], BF16, tag=f"Qs{r}{h}")
                nc.vector.tensor_copy(t[:, :qw], ps[:, :qw])
                Qs[(r, h)] = t

        if STAGE < 2 + 10 * l:
            _stub_out()
            return

        # ---- attention: ST -> exp -> A@V (+ row sums), accumulated in PSUM
        # Accumulators are zeroed by DVE memset; every matmul then uses
        # start=False (pure accumulate), so scheduler order within the
        # region doesn't matter.  skip_group_check silences the group
        # bookkeeping that this pattern sidesteps.
        OT_ps = psacc.tile([128, 2, NPOS], F32, tag="ot")   # [ch128, h, q]
        RS_ps = psacc.tile([128, 2, NPOS], F32, tag="rs")
        nc.vector.memset(OT_ps[:], 0.0)
        nc.vector.memset(RS_ps[:], 0.0)
        sc_idx = 0
        for J in range(2):          # key c'-quad
            for pc in range(2):     # key position chunk
                ATl = {}
                for h in range(2):
                    for r in range(4):
                        # each matmul gets its own 2KB psum zero-region
                        # (slices padded to 512 f32): region-sharing with a
                        # split start/stop group crashes the device.
                        psa = psw.tile([128, 2, 512], F32, tag="qk")
                        psb = psw.tile([128, 2, 512], F32, tag="qk")
                        for i in range(4):
                            pst = psa if i < 2 else psb
                            nc.tensor.matmul(
                                pst[:, i % 2, :qw],
                                KT[J][32 * i:32 * (i + 1),
                                      pc * 128:(pc + 1) * 128],
                                Qs[(r, h)][32 * i:32 * (i + 1), :qw],
                                start=True, stop=True,
                                tile_position=(32 * i, 0))
                        for half, pst in ((0, psa), (1, psb)):
                            at2 = atp.tile([128, 2, NPOS], BF16, tag="at")
                            nc.scalar.activation(at2[:, :, :qw],
                                                 pst[:, :, :qw],
                                                 AF.Exp)
                            for g in range(2):
                                i = 2 * half + g
                                c = 4 * h + (i + r) % 4
                                ATl[(c, i)] = at2[:, g, :qw]
                for i in range(4):  # s-chunk (c' = 4J+i, pc)
                    first = sc_idx == 0
                    last = sc_idx == 15
                    sc_idx += 1
                    cp = 4 * J + i
                    Vv = V[pc][:, 32 * cp:32 * (cp + 1)]   # [128, 32] bf16
                    # OT_ps / RS_ps are each exactly one 2KB zero region:
                    # start only on the very first matmul, stop on the last.
                    for h in range(2):
                        for j in range(4):
                            nc.tensor.matmul(
                                OT_ps[32 * j:32 * (j + 1), h, :qw],
                                Vv, ATl[(4 * h + j, i)],
                                start=False, stop=False,
                                skip_group_check=True,
                                tile_position=(0, 32 * j))
                        for j in range(4):
                            nc.tensor.matmul(
                                RS_ps[32 * j:32 * j + 1, h, :qw],
                                ones_col[:], ATl[(4 * h + j, i)],
                                start=False, stop=False,
                                skip_group_check=True,
                                tile_position=(0, 32 * j))

        if STAGE < 3 + 10 * l:
            _stub_out()
            return

        # ---- normalize: OT = OT * (1/RS) broadcast over the 32-row blocks
        OT_sb = {}
        for h in range(2):
            rs_sb = work.tile([128, NPOS], F32, tag="rs_sb")
            for j in range(4):
                nc.vector.reciprocal(rs_sb[32 * j:32 * j + 1, :qw],
                                     RS_ps[32 * j:32 * j + 1, h, :qw])
            rs_dr = drp.tile([4, NPOS], F32, tag="rs_dr")
            dma(out=rs_dr[:, :qw],
                in_=rs_sb.rearrange("(j d) q -> j d q", d=32)[:, 0, :qw])
            rb = work.tile([128, NPOS], F32, tag="rb")
            for j in range(4):
                nc.sync.dma_start(
                    out=rb[32 * j:32 * (j + 1), :qw],
                    in_=rs_dr[j, :qw].partition_broadcast(32))
            t = work.tile([128, NPOS], WDT, tag=f"OT{h}")
            nc.vector.tensor_tensor(t[:, :qw], OT_ps[:, h, :qw], rb[:, :qw],
                                    MULT)
            OT_sb[h] = t

        # ---- O @ Wo + bo + residual -> LN1 -> xa
        xa = {}
        for ci, (xi, ro, nr) in enumerate(pos_chunks):
            ps = psw.tile([128, 1024], F32, tag="qk")
            seed_bias(ps[:nr, :E], brows[("bo", l)][0:1, :], nr, E)
            for h in range(2):
                nc.tensor.matmul(
                    ps[:nr, :E],
                    OT_sb[h][:, ci * 128:ci * 128 + nr],
                    Wo_t[(l, h)][:],
                    start=False, stop=(h == 1))
            res = work.tile([128, NPOS], F32, tag=f"res{ci}")
            nc.vector.tensor_add(res[:nr], ps[:nr, :E],
                                 X_t[xi][ro:ro + nr, :])
            t = work.tile([128, NPOS], F32, tag=f"xa{ci}")
            layernorm(res[:nr], nr, ln_b[("ln1g", l)], ln_b[("ln1b", l)],
                      t[:nr])
            xa[ci] = t

        if STAGE < 4 + 10 * l:
            _stub_out()
            return

        # ---- transpose xa -> xaT channel-major
        xaT = {}
        nchunk = len(pos_chunks)
        for k in range(2):
            t = work.tile([128, NPOS], WDT, tag=f"xaT{k}")
            for ci, (_, _, nr) in enumerate(pos_chunks):
                ps = psw.tile([128, 1024], F32, tag="qk")
                nc.tensor.transpose(ps[:, :nr],
                                    xa[ci][:nr, k * 128:(k + 1) * 128],
                                    ident[:nr, :nr])
                nc.vector.tensor_copy(t[:, ci * 128:ci * 128 + nr],
                                      ps[:, :nr])
            xaT[k] = t

        # ---- FFN: H1T = relu(W1.T x + b1) channel-major bf16 [128, qw] x8
        H1T = {}
        for dk in range(8):
            ps = psw.tile([128, 1024], F32, tag="qk")
            for k in range(2):
                nc.tensor.matmul(
                    ps[:, :qw],
                    W1_t[(l, k)][:, dk * 128:(dk + 1) * 128],
                    xaT[k][:, :qw],
                    start=(k == 0), stop=(k == 1))
            t = work.tile([128, NPOS], BF16, tag=f"H1T{dk}")
            nc.vector.tensor_scalar(t[:, :qw], ps[:, :qw],
                                    b1_t[l][:, dk:dk + 1], 0.0, ADD, MAX)
            H1T[dk] = t

        if STAGE < 5 + 10 * l:
            _stub_out()
            return

        # ---- FF = relu(H1 @ W2 + b2); X_next = LN2(xa + FF)
        newX = {}
        for ci, (_, _, nr) in enumerate(pos_chunks):
            ps = psw.tile([128, 1024], F32, tag="qk")
            seed_bias(ps[:nr, :E], brows[("b2", l)][0:1, :], nr, E)
            for dk in range(8):
                nc.tensor.matmul(
                    ps[:nr, :E],
                    H1T[dk][:, ci * 128:ci * 128 + nr],
                    W2_t[(l, dk)][:],
                    start=False, stop=(dk == 7))
            t = work.tile([128, NPOS], F32, tag=f"ff{ci}")
            nc.vector.tensor_scalar_max(t[:nr], ps[:nr, :E], 0.0)
            res2 = work.tile([128, NPOS], F32, tag=f"res2{ci}")
            nc.vector.tensor_add(res2[:nr], t[:nr], xa[ci][:nr])
            xn = const.tile([128, NPOS], F32, tag=f"Xn{l}{ci}")
            layernorm(res2[:nr], nr, ln_b[("ln2g", l)], ln_b[("ln2b", l)],
                      xn[:nr])
            newX[ci] = xn

        if l == 0:
            X_t = {0: newX[0], 1: newX[1]}
            XT_t = {}
            for k in range(2):
                t = const.tile([128, NPOS], WDT, tag=f"X1T{k}")
                for ci in range(2):
                    ps = psw.tile([128, 1024], F32, tag="qk")
                    nc.tensor.transpose(ps[:, :128],
                                        newX[ci][:, k * 128:(k + 1) * 128],
                                        ident[:])
                    nc.vector.tensor_copy(t[:, ci * 128:(ci + 1) * 128],
                                          ps[:, :128])
                XT_t[k] = t
        else:
            X2 = newX[0]  # [96, 256]

    # ---------------- final LN + projection ----------------
    xf = work.tile([128, NPOS], F32, tag="xf")
    layernorm(X2[:128], 128, ln_b["lnfg"], ln_b["lnfb"], xf[:128])
    xfT = {}
    for k in range(2):
        ps = psw.tile([128, 1024], F32, tag="qk")
        nc.tensor.transpose(ps[:, :128], xf[:, k * 128:(k + 1) * 128],
                            ident[:])
        t = work.tile([128, 128], F32, tag=f"xfT{k}")
        nc.vector.tensor_copy(t[:], ps[:, :128])
        xfT[k] = t
    ps = psw.tile([128, 1024], F32, tag="qk")
    nc.tensor.matmul(ps[:, 0:1], ones_row[0:1, :],
                     bprow[0:1, 0:1], start=True, stop=False)
    for k in range(2):
        nc.tensor.matmul(ps[:, 0:1], xfT[k][:],
                         Wp_sb[:, k:k + 1],
                         start=False, stop=(k == 1))
    ot = work.tile([128, 1], F32, tag="outsb")
    nc.vector.tensor_copy(ot[:], ps[:, 0:1])
    # output = last 96 of the 128 computed positions
    nc.sync.dma_start(out=outs["out"][:], in_=ot[128 - PRED:, :])


# ======================= host side =======================

def _rot_matrices():
    """P_r[k, m] = 1 iff k = 32*((m//32 + r) % 4) + m % 32, r = 1..3."""
    import ml_dtypes
    mats = np.zeros((3, 128, 128), np.float32)
    for r in range(1, 4):
        for m in range(128):
            mats[r - 1, 32 * ((m // 32 + r) % 4) + m % 32, m] = 1.0
    return mats.astype(ml_dtypes.bfloat16)


def _make_in_maps(inputs):
    import ml_dtypes
    f = np.float32
    w = np.float32 if MM_DTYPE == "f32" else ml_dtypes.bfloat16
    x_enc = np.asarray(inputs["x_enc"], f)
    td = np.asarray(inputs["time_diffs"], f)
    w2bf = np.asarray(inputs["W2"], f).astype(ml_dtypes.bfloat16)

    def wa(a):  # matmul-operand array -> WDT, contiguous
        return np.ascontiguousarray(np.asarray(a, f).astype(w))

    base = {
        "Wemb": np.ascontiguousarray(inputs["W_emb"], f),
        "Wq": wa(inputs["Wq"]),
        "Wk": wa(inputs["Wk"]),
        "Wv": wa(inputs["Wv"]),
        "Wo": wa(inputs["Wo"]),
        "W1": wa(inputs["W1"]),
        "W2bf": np.ascontiguousarray(w2bf),
        "bq": np.ascontiguousarray(inputs["bq"], f),
        "bk": np.ascontiguousarray(inputs["bk"], f),
        "bv": wa(inputs["bv"]),
        "bo": wa(inputs["bo"]),
        "b2": wa(inputs["b2"]),
        "b1": np.ascontiguousarray(inputs["b1"], f),
        "bemb": np.ascontiguousarray(inputs["b_emb"], f),
        "bembw": wa(inputs["b_emb"]),
        "ln1g": np.ascontiguousarray(inputs["ln1_g"], f),
        "ln1b": np.ascontiguousarray(inputs["ln1_b"], f),
        "ln2g": np.ascontiguousarray(inputs["ln2_g"], f),
        "ln2b": np.ascontiguousarray(inputs["ln2_b"], f),
        "lnfg": np.ascontiguousarray(inputs["lnf_g"], f),
        "lnfb": np.ascontiguousarray(inputs["lnf_b"], f),
        "Wp2": np.ascontiguousarray(np.asarray(inputs["W_proj"], f)[:, 0].reshape(2, 128).T),
        "bproj": np.asarray(inputs["b_proj"], f)[0].reshape(1, 1),
        "Prot": _rot_matrices(),
    }
    maps = []
    for b in range(B):
        m = dict(base)
        m["xT"] = np.ascontiguousarray(x_enc[b, P0:P0 + NPOS, :].T)
        m["td"] = np.ascontiguousarray(td[b:b + 1, :])
        maps.append(m)
    return maps


_CACHE = {}


def _run(in_maps, check_with_sim=False, check_with_hw=True, **kw):
    from concourse.bass_test_utils import run_kernel

    n = len(in_maps)
    out_like = {"out": np.zeros((PRED, 1), np.float32)}
    res = run_kernel(
        lambda tc, outs, ins: chaos_kernel(tc, outs, ins),
        None,
        in_maps if n > 1 else in_maps[0],
        output_like=[out_like] * n if n > 1 else out_like,
        bass_type=tile.TileContext,
        num_cores=n,
        check_with_sim=check_with_sim,
        check_with_hw=check_with_hw,
        trace_sim=False,
        **kw,
    )
    return res


def kernel(**inputs):
    in_maps = _make_in_maps(inputs)
    res = _run(in_maps)
    out = np.stack(
        [list(res.results[b].values())[0].reshape(PRED) for b in range(B)])
    return out.astype(np.float32)



# revision 4
# speedup vs baseline: 1.3631x; 1.3631x over previous
"""Trainium2 Bass kernel for nn_ChaosTransformer_22333829939822.

Key mathematical reduction (verified against the reference):
the torch-style ``view(B, H, L, E//H)`` on a [B, L, E] tensor is a raw
row-major reshape, which makes head h attend only within the 256-position
block [h*256, (h+1)*256).  The output ``dec[:, -96:, 0]`` therefore depends
only on the last 256 positions of each batch.  Each core runs one batch's
[256, 256] residual-stream transformer; attention operates on the
[2048, 32] head-view of the 256x256 block.

Sharding: data-parallel over batch B across 4 of the 8 cores (one batch
per core, fully independent, no collectives).

v1 performance rework (from the 276us baseline trace):
- all weights ship in one bf16 blob + one small f32 blob (few large DMAs
  at fabric rate instead of ~100 descriptor-bound transfers).
- decay tiles D[h][32j+d, q] and all rearranged biases precomputed on host
  (the on-device build was 65k 4-byte DMA packets = 45us of dead time).
- LN gain/bias broadcast tiles built by the idle GpSimd engine.
- softmax row sums accumulated as M=32 ones-matmuls (same PE cost as M=1,
  yields the 32-row broadcast for free); 1/RS via reciprocal_approx_fast.
- score matmuls stream N=512 (2-4 score tiles per matmul) into 2-bank PSUM
  regions; ONE exp ACTIVATE per region amortizes ACT's 352-cycle fixed
  overhead (exp is the bottleneck engine: 1 elem/cyc/lane @ 1.2 GHz).
"""

import sys
import numpy as np

sys.path.insert(0, "/opt/trn_rl_repo")

import concourse.bass as bass
import concourse.tile as tile
from concourse import mybir
from concourse.masks import make_identity

F32 = mybir.dt.float32
BF16 = mybir.dt.bfloat16
WDT = BF16
ADD = mybir.AluOpType.add
SUB = mybir.AluOpType.subtract
MULT = mybir.AluOpType.mult
MAX = mybir.AluOpType.max
AF = mybir.ActivationFunctionType

B, L, D, E, DFF, LYR, PRED = 4, 2048, 7, 256, 1024, 2, 96
FACTOR = 5.0
SCALE = 1.0 / float(np.sqrt(FACTOR))
EPS = 1e-5
P0 = L - 256          # 1792: start of the last 256-position block
QLO2 = 128            # layer-2 computes query positions [128, 256)
NPOS = 256


# ---------------- blob layouts (host + device share these) ----------------

def _bf16_layout():
    """Column offsets into the [128, C] bf16 weight blob."""
    off = {}
    c = 0
    def put(key, w):
        nonlocal c
        off[key] = c
        c += w
    for r in range(3):
        put(("Prot", r), 128)
    for l in range(LYR):
        for k in range(2):
            put(("Wq", l, k), E)
            put(("Wk", l, k), E)
            put(("Wv", l, k), E)
        for h in range(2):
            put(("Wo", l, h), E)
        for k in range(2):
            put(("W1", l, k), DFF)
        for dk in range(8):
            put(("W2", l, dk), E)
    return off, c


def _f32_layout():
    off = {}
    c = 0
    def put(key, w):
        nonlocal c
        off[key] = c
        c += w
    for h in range(2):
        put(("D", h), NPOS)
    for l in range(LYR):
        put(("bq", l), 2)
        put(("bk", l), 2)
        put(("b1", l), 8)
    put("bemb_pp", 2)
    put("Wp2", 2)
    return off, c


def _rows_layout():
    """f32 rows on partition 0: LN rows + f32 bias rows + bproj."""
    off = {}
    c = 0
    def put(key, w):
        nonlocal c
        off[key] = c
        c += w
    for nm in ("ln1g", "ln1b", "ln2g", "ln2b"):
        for l in range(LYR):
            put((nm, l), E)
    put("lnfg", E)
    put("lnfb", E)
    put("bemb_r", E)
    put("bproj", 1)
    return off, c


def _wrows_layout():
    """bf16 bias rows on partition 0 (seed rows for psum bias init)."""
    off = {}
    c = 0
    def put(key, w):
        nonlocal c
        off[key] = c
        c += w
    for nm in ("bv", "bo", "b2"):
        for l in range(LYR):
            put((nm, l), E)
    return off, c


BF_OFF, BF_COLS = _bf16_layout()
F_OFF, F_COLS = _f32_layout()
R_OFF, R_COLS = _rows_layout()
WR_OFF, WR_COLS = _wrows_layout()


def chaos_kernel(tc, outs, ins):
    import contextlib

    nc = tc.nc
    with contextlib.ExitStack() as ctx:
        _chaos_body(tc, nc, ctx, outs, ins)


def _chaos_body(tc, nc, ctx, outs, ins):
    const = ctx.enter_context(tc.tile_pool(name="const", bufs=1))
    work = ctx.enter_context(tc.tile_pool(name="work", bufs=2))
    atp = ctx.enter_context(tc.tile_pool(name="atp", bufs=12))
    psw = ctx.enter_context(tc.tile_pool(name="psw", bufs=2, space="PSUM"))
    scp = ctx.enter_context(tc.tile_pool(name="scp", bufs=2, space="PSUM"))
    psacc = ctx.enter_context(tc.tile_pool(name="psacc", bufs=1, space="PSUM"))

    dma = nc.sync.dma_start

    # ---------------- constant loads (few big DMAs) ----------------
    blob = const.tile([128, BF_COLS], BF16, tag="blob")
    # split: layer-0 weights (+Prot) first so compute can start early
    split = BF_OFF[("Wq", 1, 0)]
    dma(out=blob[:, :split], in_=ins["blob"][:, :split])
    dma(out=blob[:, split:], in_=ins["blob"][:, split:])
    fblob = const.tile([128, F_COLS], F32, tag="fblob")
    dma(out=fblob[:], in_=ins["fblob"][:])
    rows = const.tile([1, R_COLS], F32, tag="rows")
    dma(out=rows[:], in_=ins["rows"][:])
    wrows = const.tile([1, WR_COLS], BF16, tag="wrows")
    dma(out=wrows[:], in_=ins["wrows"][:])
    xw = const.tile([D, 2 * NPOS], F32, tag="xw")
    dma(out=xw[:], in_=ins["xw"][:])
    xT_sb = xw[:, :NPOS]
    Wemb_sb = xw[:, NPOS:]

    def bf(key):
        w = {"Prot": 128, "W1": DFF}.get(key[0], E)
        return blob[:, BF_OFF[key]:BF_OFF[key] + w]

    def fb(key):
        w = {"D": NPOS, "b1": 8}.get(key[0] if isinstance(key, tuple) else key, 2)
        return fblob[:, F_OFF[key]:F_OFF[key] + w]

    def rrow(key):
        w = 1 if key == "bproj" else E
        return rows[0:1, R_OFF[key]:R_OFF[key] + w]

    def wrow(key):
        return wrows[0:1, WR_OFF[key]:WR_OFF[key] + E]

    # LN gain/bias broadcast tiles via GpSimd (engine is otherwise idle)
    ln_b = {}
    for nm in ("ln1g", "ln1b", "ln2g", "ln2b"):
        for l in range(LYR):
            t = const.tile([128, E], F32, tag=f"{nm}{l}")
            nc.gpsimd.partition_broadcast(t[:], rrow((nm, l)))
            ln_b[(nm, l)] = t
    for nm in ("lnfg", "lnfb"):
        t = const.tile([128, E], F32, tag=nm)
        nc.gpsimd.partition_broadcast(t[:], rrow(nm))
        ln_b[nm] = t

    ident = const.tile([128, 128], F32, tag="ident")
    make_identity(nc, ident[:])
    ones_col = const.tile([128, 1], BF16, tag="ones_col")
    nc.vector.memset(ones_col[:], 1.0)
    ones32 = const.tile([128, 32], BF16, tag="ones32")
    nc.vector.memset(ones32[:], 1.0)
    ones_row = const.tile([1, 128], F32, tag="ones_row")
    nc.vector.memset(ones_row[:], 1.0)
    ones_row_w = const.tile([1, 128], WDT, tag="ones_row_w")
    nc.vector.memset(ones_row_w[:], 1.0)
    eps_t = const.tile([128, 1], F32, tag="eps")
    nc.vector.memset(eps_t[:], EPS)

    def seed_bias(ps_ap, brow_ap, m):
        """PSUM <- bias row broadcast over m partitions (K=1 matmul)."""
        ones = ones_row if brow_ap.dtype == F32 else ones_row_w
        nc.tensor.matmul(ps_ap, ones[0:1, :m], brow_ap, start=True, stop=False)

    def layernorm(x_ap, rows_n, g_b, b_b, out_ap):
        st = work.tile([128, 6], F32, tag="bn_st")
        nc.vector.bn_stats(st[:rows_n], x_ap)
        mv = work.tile([128, 2], F32, tag="bn_mv")
        nc.vector.bn_aggr(mv[:rows_n], st[:rows_n])
        sd = work.tile([128, 1], F32, tag="bn_sd")
        nc.scalar.activation(sd[:rows_n], mv[:rows_n, 1:2], AF.Sqrt,
                             bias=eps_t[:rows_n])
        nc.vector.reciprocal(sd[:rows_n], sd[:rows_n])
        t = work.tile([128, NPOS], F32, tag="ln_t")
        nc.vector.tensor_scalar(t[:rows_n], x_ap, mv[:rows_n, 0:1], sd[:rows_n],
                                SUB, MULT)
        nc.vector.tensor_mul(t[:rows_n], t[:rows_n], g_b[:rows_n])
        nc.vector.tensor_add(out_ap, t[:rows_n], b_b[:rows_n])

    # ---------------- embedding ----------------
    X_t, XT_t = {}, {}
    for p in range(2):  # position-major X
        ps = psw.tile([128, 512], F32, tag="qk")
        seed_bias(ps[:, :E], rrow("bemb_r"), 128)
        nc.tensor.matmul(ps[:, :E], xT_sb[:, p * 128:(p + 1) * 128],
                         Wemb_sb[:], start=False, stop=True)
        t = const.tile([128, NPOS], F32, tag=f"X{p}")
        nc.vector.tensor_copy(t[:], ps[:, :E])
        X_t[p] = t
    for k in range(2):  # channel-major XT
        ps = psw.tile([128, 512], F32, tag="qk")
        nc.tensor.matmul(ps[:, :NPOS], Wemb_sb[:, k * 128:(k + 1) * 128],
                         xT_sb[:], start=True, stop=True)
        t = const.tile([128, NPOS], WDT, tag=f"XT{k}")
        nc.vector.tensor_scalar_add(t[:], ps[:, :NPOS],
                                    fb("bemb_pp")[:, k:k + 1])
        XT_t[k] = t

    # ---------------- transformer layers ----------------
    for l in range(LYR):
        qlo, qhi = (0, NPOS) if l == 0 else (QLO2, NPOS)
        qw = qhi - qlo
        pos_chunks = ([(0, 0, 128), (1, 0, 128)] if l == 0
                      else [(1, 0, 128)])
        # (X-tile index, row offset within tile, nrows) for output positions

        # ---- K projection -> KT channel-major bf16 [128, 256] x2
        KT = {}
        for Jt in range(2):
            ps = psw.tile([128, 512], F32, tag="qk")
            for k in range(2):
                nc.tensor.matmul(
                    ps[:, :NPOS],
                    bf(("Wk", l, k))[:, Jt * 128:(Jt + 1) * 128],
                    XT_t[k][:], start=(k == 0), stop=(k == 1))
            t = work.tile([128, NPOS], BF16, tag=f"KT{Jt}")
            nc.vector.tensor_scalar_add(t[:], ps[:, :NPOS],
                                        fb(("bk", l))[:, Jt:Jt + 1])
            KT[Jt] = t

        # ---- V projection -> V position-major bf16 [128, 256] x2
        V = {}
        for pc in range(2):
            ps = psw.tile([128, 512], F32, tag="qk")
            seed_bias(ps[:, :E], wrow(("bv", l)), 128)
            for k in range(2):
                nc.tensor.matmul(
                    ps[:, :E], XT_t[k][:, pc * 128:(pc + 1) * 128],
                    bf(("Wv", l, k))[:], start=False, stop=(k == 1))
            t = work.tile([128, E], BF16, tag=f"V{pc}")
            nc.vector.tensor_copy(t[:], ps[:, :E])
            V[pc] = t

        # ---- Q projection -> Qs_all flat [128, 8*qw] bf16
        # slot s = 4h + r at cols [s*qw, (s+1)*qw); rows 32i hold q-chunk
        # c = 4h + (i + r) % 4 (decay-scaled).
        Qs_all = work.tile([128, 8 * qw], BF16, tag=f"qsall{l}")
        for h in range(2):
            ps = psw.tile([128, 512], F32, tag="qk")
            for k in range(2):
                nc.tensor.matmul(
                    ps[:, :qw],
                    bf(("Wq", l, k))[:, h * 128:(h + 1) * 128],
                    XT_t[k][:, qlo:qhi], start=(k == 0), stop=(k == 1))
            tf = work.tile([128, NPOS], F32, tag="qtmp")
            nc.vector.tensor_scalar_add(tf[:, :qw], ps[:, :qw],
                                        fb(("bq", l))[:, h:h + 1])
            nc.vector.tensor_mul(Qs_all[:, 4 * h * qw:(4 * h + 1) * qw],
                                 tf[:, :qw], fb(("D", h))[:, qlo:qhi])
        for r in range(1, 4):
            for h in range(2):
                ps = psw.tile([128, 512], F32, tag="qk")
                nc.tensor.matmul(ps[:, :qw], bf(("Prot", r - 1))[:],
                                 Qs_all[:, 4 * h * qw:(4 * h + 1) * qw],
                                 start=True, stop=True)
                nc.vector.tensor_copy(
                    Qs_all[:, (4 * h + r) * qw:(4 * h + r + 1) * qw],
                    ps[:, :qw])

        # ---- attention: ST -> exp -> A@V (+ row sums), accumulated in PSUM
        # OT_ps[32j+d, h, q]: attention out for q-chunk c=4h+j, head dim d.
        # RS_ps[32j+d, h, q]: softmax denominator for q-chunk c=4h+j (the
        # M=32 ones matmul broadcasts the row over d for free).
        # Accumulators are zeroed by DVE memset; every matmul then uses
        # start=False (pure accumulate).
        OT_ps = psacc.tile([128, 2, NPOS], F32, tag="ot")
        RS_ps = psacc.tile([128, 2, NPOS], F32, tag="rs")
        nc.vector.memset(OT_ps[:], 0.0)
        nc.vector.memset(RS_ps[:], 0.0)
        # Each score matmul streams N=512 moving cols (= 512//qw score
        # tiles) into one PSUM bank; a 2-bank region gets ONE exp call
        # (amortizes ACT's 352-cycle fixed overhead).
        for J in range(2):          # key c'-quad
            for pc in range(2):     # key position chunk
                ATl = {}
                for i in range(4):
                    if qw == NPOS:
                        # L1: one region per (i, h); bank g <- r-pair
                        # (2g, 2g+1), tile r at cols (r%2)*qw.
                        for h in range(2):
                            sc = scp.tile([128, 2, 512], F32, tag="sc")
                            at = atp.tile([128, 2, 512], BF16, tag="at")
                            for g in range(2):
                                nc.tensor.matmul(
                                    sc[:, g, :],
                                    KT[J][32 * i:32 * (i + 1),
                                          pc * 128:(pc + 1) * 128],
                                    Qs_all[32 * i:32 * (i + 1),
                                           (4 * h + 2 * g) * qw:
                                           (4 * h + 2 * g + 2) * qw],
                                    start=True, stop=True,
                                    tile_position=(32 * i, 0))
                            nc.scalar.activation(at[:], sc[:], AF.Exp)
                            for r in range(4):
                                c = 4 * h + (i + r) % 4
                                ATl[(c, i)] = at[:, r // 2,
                                                 (r % 2) * qw:(r % 2 + 1) * qw]
                    else:
                        # L2 (qw=128): one region per i; bank h <- all 4
                        # r's of that h, tile r at cols r*qw.
                        sc = scp.tile([128, 2, 512], F32, tag="sc")
                        at = atp.tile([128, 2, 512], BF16, tag="at")
                        for h in range(2):
                            nc.tensor.matmul(
                                sc[:, h, :],
                                KT[J][32 * i:32 * (i + 1),
                                      pc * 128:(pc + 1) * 128],
                                Qs_all[32 * i:32 * (i + 1),
                                       4 * h * qw:(4 * h + 4) * qw],
                                start=True, stop=True,
                                tile_position=(32 * i, 0))
                        nc.scalar.activation(at[:], sc[:], AF.Exp)
                        for h in range(2):
                            for r in range(4):
                                c = 4 * h + (i + r) % 4
                                ATl[(c, i)] = at[:, h, r * qw:(r + 1) * qw]
                for i in range(4):  # s-chunk (c' = 4J+i, pc)
                    cp = 4 * J + i
                    Vv = V[pc][:, 32 * cp:32 * (cp + 1)]   # [128, 32] bf16
                    for h in range(2):
                        for j in range(4):
                            nc.tensor.matmul(
                                OT_ps[32 * j:32 * (j + 1), h, :qw],
                                Vv, ATl[(4 * h + j, i)],
                                start=False, stop=False,
                                skip_group_check=True,
                                tile_position=(0, 32 * j))
                        for j in range(4):
                            nc.tensor.matmul(
                                RS_ps[32 * j:32 * (j + 1), h, :qw],
                                ones32[:], ATl[(4 * h + j, i)],
                                start=False, stop=False,
                                skip_group_check=True,
                                tile_position=(0, 32 * j))

        # ---- normalize: OT = OT * (1/RS)
        rinv = work.tile([128, 2, NPOS], F32, tag="rinv")
        nc.vector.reciprocal_approx_fast(rinv[:, :, :qw], RS_ps[:, :, :qw])
        OT_sb = {}
        for h in range(2):
            t = work.tile([128, NPOS], WDT, tag=f"OT{h}")
            nc.vector.tensor_tensor(t[:, :qw], OT_ps[:, h, :qw],
                                    rinv[:, h, :qw], MULT)
            OT_sb[h] = t

        # ---- O @ Wo + bo + residual -> LN1 -> xa
        xa = {}
        for ci, (xi, ro, nr) in enumerate(pos_chunks):
            ps = psw.tile([128, 512], F32, tag="qk")
            seed_bias(ps[:nr, :E], wrow(("bo", l)), nr)
            for h in range(2):
                nc.tensor.matmul(
                    ps[:nr, :E], OT_sb[h][:, ci * 128:ci * 128 + nr],
                    bf(("Wo", l, h))[:], start=False, stop=(h == 1))
            res = work.tile([128, NPOS], F32, tag=f"res{ci}")
            nc.vector.tensor_add(res[:nr], ps[:nr, :E],
                                 X_t[xi][ro:ro + nr, :])
            t = work.tile([128, NPOS], F32, tag=f"xa{ci}")
            layernorm(res[:nr], nr, ln_b[("ln1g", l)], ln_b[("ln1b", l)],
                      t[:nr])
            xa[ci] = t

        # ---- transpose xa -> xaT channel-major
        xaT = {}
        for k in range(2):
            t = work.tile([128, NPOS], WDT, tag=f"xaT{k}")
            for ci, (_, _, nr) in enumerate(pos_chunks):
                ps = psw.tile([128, 512], F32, tag="qk")
                nc.tensor.transpose(ps[:, :nr],
                                    xa[ci][:nr, k * 128:(k + 1) * 128],
                                    ident[:nr, :nr])
                nc.vector.tensor_copy(t[:, ci * 128:ci * 128 + nr],
                                      ps[:, :nr])
            xaT[k] = t

        # ---- FFN: H1T = relu(W1.T x + b1) channel-major bf16 [128, qw] x8
        H1T = {}
        for dk in range(8):
            ps = psw.tile([128, 512], F32, tag="qk")
            for k in range(2):
                nc.tensor.matmul(
                    ps[:, :qw],
                    bf(("W1", l, k))[:, dk * 128:(dk + 1) * 128],
                    xaT[k][:, :qw], start=(k == 0), stop=(k == 1))
            t = work.tile([128, NPOS], BF16, tag=f"H1T{dk}")
            nc.vector.tensor_scalar(t[:, :qw], ps[:, :qw],
                                    fb(("b1", l))[:, dk:dk + 1], 0.0,
                                    ADD, MAX)
            H1T[dk] = t

        # ---- FF = relu(H1 @ W2 + b2); X_next = LN2(xa + FF)
        newX = {}
        for ci, (_, _, nr) in enumerate(pos_chunks):
            ps = psw.tile([128, 512], F32, tag="qk")
            seed_bias(ps[:nr, :E], wrow(("b2", l)), nr)
            for dk in range(8):
                nc.tensor.matmul(
                    ps[:nr, :E], H1T[dk][:, ci * 128:ci * 128 + nr],
                    bf(("W2", l, dk))[:], start=False, stop=(dk == 7))
            t = work.tile([128, NPOS], F32, tag=f"ff{ci}")
            nc.vector.tensor_scalar_max(t[:nr], ps[:nr, :E], 0.0)
            res2 = work.tile([128, NPOS], F32, tag=f"res2{ci}")
            nc.vector.tensor_add(res2[:nr], t[:nr], xa[ci][:nr])
            xn = const.tile([128, NPOS], F32, tag=f"Xn{l}{ci}")
            layernorm(res2[:nr], nr, ln_b[("ln2g", l)], ln_b[("ln2b", l)],
                      xn[:nr])
            newX[ci] = xn

        if l == 0:
            X_t = {0: newX[0], 1: newX[1]}
            XT_t = {}
            for k in range(2):
                t = const.tile([128, NPOS], WDT, tag=f"X1T{k}")
                for ci in range(2):
                    ps = psw.tile([128, 512], F32, tag="qk")
                    nc.tensor.transpose(ps[:, :128],
                                        newX[ci][:, k * 128:(k + 1) * 128],
                                        ident[:])
                    nc.vector.tensor_copy(t[:, ci * 128:(ci + 1) * 128],
                                          ps[:, :128])
                XT_t[k] = t
        else:
            X2 = newX[0]  # [128, 256]

    # ---------------- final LN + projection ----------------
    xf = work.tile([128, NPOS], F32, tag="xf")
    layernorm(X2[:128], 128, ln_b["lnfg"], ln_b["lnfb"], xf[:128])
    xfT = {}
    for k in range(2):
        ps = psw.tile([128, 512], F32, tag="qk")
        nc.tensor.transpose(ps[:, :128], xf[:, k * 128:(k + 1) * 128],
                            ident[:])
        t = work.tile([128, 128], F32, tag=f"xfT{k}")
        nc.vector.tensor_copy(t[:], ps[:, :128])
        xfT[k] = t
    ps = psw.tile([128, 512], F32, tag="qk")
    nc.tensor.matmul(ps[:, 0:1], ones_row[0:1, :], rrow("bproj"),
                     start=True, stop=False)
    for k in range(2):
        nc.tensor.matmul(ps[:, 0:1], xfT[k][:], fb("Wp2")[:, k:k + 1],
                         start=False, stop=(k == 1))
    ot = work.tile([128, 1], F32, tag="outsb")
    nc.vector.tensor_copy(ot[:], ps[:, 0:1])
    # output = last 96 of the 128 computed positions
    nc.sync.dma_start(out=outs["out"][:], in_=ot[128 - PRED:, :])


# ======================= host side =======================

def _rot_matrices():
    """P_r[k, m] = 1 iff k = 32*((m//32 + r) % 4) + m % 32, r = 1..3."""
    mats = np.zeros((3, 128, 128), np.float32)
    for r in range(1, 4):
        for m in range(128):
            mats[r - 1, 32 * ((m // 32 + r) % 4) + m % 32, m] = 1.0
    return mats


def _make_in_maps(inputs):
    import ml_dtypes
    f = np.float32
    bh = ml_dtypes.bfloat16
    x_enc = np.asarray(inputs["x_enc"], f)
    td = np.asarray(inputs["time_diffs"], f)

    blob = np.zeros((128, BF_COLS), bh)
    rot = _rot_matrices()
    for r in range(3):
        blob[:, BF_OFF[("Prot", r)]:BF_OFF[("Prot", r)] + 128] = rot[r]
    for l in range(LYR):
        for nm in ("Wq", "Wk", "Wv"):
            w = np.asarray(inputs[nm], f)[l]
            for k in range(2):
                blob[:, BF_OFF[(nm, l, k)]:BF_OFF[(nm, l, k)] + E] = \
                    w[k * 128:(k + 1) * 128, :]
        wo = np.asarray(inputs["Wo"], f)[l]
        for h in range(2):
            blob[:, BF_OFF[("Wo", l, h)]:BF_OFF[("Wo", l, h)] + E] = \
                wo[h * 128:(h + 1) * 128, :]
        w1 = np.asarray(inputs["W1"], f)[l]
        for k in range(2):
            blob[:, BF_OFF[("W1", l, k)]:BF_OFF[("W1", l, k)] + DFF] = \
                w1[k * 128:(k + 1) * 128, :]
        w2 = np.asarray(inputs["W2"], f)[l]
        for dk in range(8):
            blob[:, BF_OFF[("W2", l, dk)]:BF_OFF[("W2", l, dk)] + E] = \
                w2[dk * 128:(dk + 1) * 128, :]

    fblob_base = np.zeros((128, F_COLS), f)
    for l in range(LYR):
        for nm, w in (("bq", 2), ("bk", 2), ("b1", 8)):
            arr = np.asarray(inputs[nm], f)[l].reshape(w, 128).T
            fblob_base[:, F_OFF[(nm, l)]:F_OFF[(nm, l)] + w] = arr
    fblob_base[:, F_OFF["bemb_pp"]:F_OFF["bemb_pp"] + 2] = \
        np.asarray(inputs["b_emb"], f).reshape(2, 128).T
    fblob_base[:, F_OFF["Wp2"]:F_OFF["Wp2"] + 2] = \
        np.asarray(inputs["W_proj"], f)[:, 0].reshape(2, 128).T

    rows = np.zeros((1, R_COLS), f)
    for nm, src in (("ln1g", "ln1_g"), ("ln1b", "ln1_b"),
                    ("ln2g", "ln2_g"), ("ln2b", "ln2_b")):
        for l in range(LYR):
            rows[0, R_OFF[(nm, l)]:R_OFF[(nm, l)] + E] = \
                np.asarray(inputs[src], f)[l]
    rows[0, R_OFF["lnfg"]:R_OFF["lnfg"] + E] = np.asarray(inputs["lnf_g"], f)
    rows[0, R_OFF["lnfb"]:R_OFF["lnfb"] + E] = np.asarray(inputs["lnf_b"], f)
    rows[0, R_OFF["bemb_r"]:R_OFF["bemb_r"] + E] = \
        np.asarray(inputs["b_emb"], f)
    rows[0, R_OFF["bproj"]] = np.asarray(inputs["b_proj"], f)[0]

    wrows = np.zeros((1, WR_COLS), bh)
    for nm, src in (("bv", "bv"), ("bo", "bo"), ("b2", "b2")):
        for l in range(LYR):
            wrows[0, WR_OFF[(nm, l)]:WR_OFF[(nm, l)] + E] = \
                np.asarray(inputs[src], f)[l]

    maps = []
    for b in range(B):
        fblob = fblob_base.copy()
        # decay tiles: D[h][32j+d, q] = SCALE * exp(-td[q*8 + 4h+j] / F)
        dec = SCALE * np.exp(-td[b].reshape(NPOS, 8) / FACTOR)  # [q, chunk]
        for h in range(2):
            tile_ = np.repeat(dec[:, 4 * h:4 * h + 4].T, 32, axis=0)
            fblob[:, F_OFF[("D", h)]:F_OFF[("D", h)] + NPOS] = tile_
        xwa = np.zeros((D, 2 * NPOS), f)
        xwa[:, :NPOS] = x_enc[b, P0:P0 + NPOS, :].T
        xwa[:, NPOS:] = np.asarray(inputs["W_emb"], f)
        maps.append({
            "blob": blob, "fblob": np.ascontiguousarray(fblob),
            "rows": rows, "wrows": wrows,
            "xw": np.ascontiguousarray(xwa),
        })
    return maps


def _run(in_maps, check_with_sim=False, check_with_hw=True, **kw):
    from concourse.bass_test_utils import run_kernel

    n = len(in_maps)
    out_like = {"out": np.zeros((PRED, 1), np.float32)}
    res = run_kernel(
        lambda tc, outs, ins: chaos_kernel(tc, outs, ins),
        None,
        in_maps if n > 1 else in_maps[0],
        output_like=[out_like] * n if n > 1 else out_like,
        bass_type=tile.TileContext,
        num_cores=n,
        check_with_sim=check_with_sim,
        check_with_hw=check_with_hw,
        trace_sim=False,
        **kw,
    )
    return res


def kernel(**inputs):
    in_maps = _make_in_maps(inputs)
    res = _run(in_maps)
    out = np.stack(
        [list(res.results[b].values())[0].reshape(PRED) for b in range(B)])
    return out.astype(np.float32)


# revision 16
# speedup vs baseline: 1.3962x; 1.0243x over previous
"""Trainium2 Bass kernel for nn_ChaosTransformer_22333829939822.

Key mathematical reduction (verified against the reference):
the torch-style ``view(B, H, L, E//H)`` on a [B, L, E] tensor is a raw
row-major reshape, which makes head h attend only within the 256-position
block [h*256, (h+1)*256).  The output ``dec[:, -96:, 0]`` therefore depends
only on the last 256 positions of each batch.  Each core runs one batch's
[256, 256] residual-stream transformer; attention operates on the
[2048, 32] head-view of the 256x256 block.

Sharding: data-parallel over batch B across 4 of the 8 cores (one batch
per core, fully independent, no collectives).

v1 performance rework (from the 276us baseline trace):
- all weights ship in one bf16 blob + one small f32 blob (few large DMAs
  at fabric rate instead of ~100 descriptor-bound transfers).
- decay tiles D[h][32j+d, q] and all rearranged biases precomputed on host
  (the on-device build was 65k 4-byte DMA packets = 45us of dead time).
- LN gain/bias broadcast tiles built by the idle GpSimd engine.
- softmax row sums accumulated as M=32 ones-matmuls (same PE cost as M=1,
  yields the 32-row broadcast for free); 1/RS via reciprocal_approx_fast.
- score matmuls stream N=512 (2-4 score tiles per matmul) into 2-bank PSUM
  regions; ONE exp ACTIVATE per region amortizes ACT's 352-cycle fixed
  overhead (exp is the bottleneck engine: 1 elem/cyc/lane @ 1.2 GHz).
"""

import sys
import numpy as np

sys.path.insert(0, "/opt/trn_rl_repo")

import concourse.bass as bass
import concourse.tile as tile
from concourse import mybir
from concourse.masks import make_identity

F32 = mybir.dt.float32
BF16 = mybir.dt.bfloat16
WDT = BF16
ADD = mybir.AluOpType.add
SUB = mybir.AluOpType.subtract
MULT = mybir.AluOpType.mult
MAX = mybir.AluOpType.max
AF = mybir.ActivationFunctionType

B, L, D, E, DFF, LYR, PRED = 4, 2048, 7, 256, 1024, 2, 96
FACTOR = 5.0
SCALE = 1.0 / float(np.sqrt(FACTOR))
EPS = 1e-5
P0 = L - 256          # 1792: start of the last 256-position block
QLO2 = 128            # layer-2 computes query positions [128, 256)
NPOS = 256


# ---------------- blob layouts (host + device share these) ----------------

def _bf16_layout():
    """Column offsets into the [128, C] bf16 weight blob."""
    off = {}
    c = 0
    def put(key, w):
        nonlocal c
        off[key] = c
        c += w
    for r in range(3):
        put(("Prot", r), 128)
    for l in range(LYR):
        for k in range(2):
            put(("Wq", l, k), E)
            put(("Wk", l, k), E)
            put(("Wv", l, k), E)
        for h in range(2):
            put(("Wo", l, h), E)
        for k in range(2):
            put(("W1", l, k), DFF)
        for dk in range(8):
            put(("W2", l, dk), E)
    return off, c


def _f32_layout():
    off = {}
    c = 0
    def put(key, w):
        nonlocal c
        off[key] = c
        c += w
    for h in range(2):
        put(("D", h), NPOS)
    for l in range(LYR):
        put(("bq", l), 2)
        put(("bk", l), 2)
        put(("b1", l), 8)
    put("bemb_pp", 2)
    put("Wp2", 2)
    return off, c


def _rows_layout():
    """f32 rows on partition 0: LN rows + f32 bias rows + bproj."""
    off = {}
    c = 0
    def put(key, w):
        nonlocal c
        off[key] = c
        c += w
    for nm in ("ln1g", "ln1b", "ln2g", "ln2b"):
        for l in range(LYR):
            put((nm, l), E)
    put("bemb_r", E)
    put("bproj", 1)
    return off, c


def _wrows_layout():
    """bf16 bias rows on partition 0 (seed rows for psum bias init)."""
    off = {}
    c = 0
    def put(key, w):
        nonlocal c
        off[key] = c
        c += w
    for nm in ("bv", "bo", "b2"):
        for l in range(LYR):
            put((nm, l), E)
    return off, c


BF_OFF, BF_COLS = _bf16_layout()
F_OFF, F_COLS = _f32_layout()
R_OFF, R_COLS = _rows_layout()
WR_OFF, WR_COLS = _wrows_layout()


def chaos_kernel(tc, outs, ins):
    import contextlib

    nc = tc.nc
    with contextlib.ExitStack() as ctx:
        _chaos_body(tc, nc, ctx, outs, ins)


def _chaos_body(tc, nc, ctx, outs, ins):
    const = ctx.enter_context(tc.tile_pool(name="const", bufs=1))
    work = ctx.enter_context(tc.tile_pool(name="work", bufs=2))
    atp = ctx.enter_context(tc.tile_pool(name="atp", bufs=12))
    psw = ctx.enter_context(tc.tile_pool(name="psw", bufs=2, space="PSUM"))
    scp = ctx.enter_context(tc.tile_pool(name="scp", bufs=2, space="PSUM"))
    psacc = ctx.enter_context(tc.tile_pool(name="psacc", bufs=1, space="PSUM"))

    dma = nc.sync.dma_start

    # ---------------- constant loads (few big DMAs) ----------------
    fblob = const.tile([128, F_COLS], F32, tag="fblob")
    dma(out=fblob[:], in_=ins["fblob"][:])
    rows = const.tile([1, R_COLS], F32, tag="rows")
    dma(out=rows[:], in_=ins["rows"][:])
    wrows = const.tile([1, WR_COLS], BF16, tag="wrows")
    dma(out=wrows[:], in_=ins["wrows"][:])
    xw = const.tile([D, 2 * NPOS], F32, tag="xw")
    dma(out=xw[:], in_=ins["xw"][:])
    blob = const.tile([128, BF_COLS], BF16, tag="blob")
    # split: layer-0 weights (+Prot) first so compute can start early
    split = BF_OFF[("Wq", 1, 0)]
    dma(out=blob[:, :split], in_=ins["blob"][:, :split])
    dma(out=blob[:, split:], in_=ins["blob"][:, split:])
    xT_sb = xw[:, :NPOS]
    Wemb_sb = xw[:, NPOS:]

    def bf(key):
        w = {"Prot": 128, "W1": DFF}.get(key[0], E)
        return blob[:, BF_OFF[key]:BF_OFF[key] + w]

    def fb(key):
        w = {"D": NPOS, "b1": 8}.get(key[0] if isinstance(key, tuple) else key, 2)
        return fblob[:, F_OFF[key]:F_OFF[key] + w]

    def rrow(key):
        w = 1 if key == "bproj" else E
        return rows[0:1, R_OFF[key]:R_OFF[key] + w]

    def wrow(key):
        return wrows[0:1, WR_OFF[key]:WR_OFF[key] + E]

    # LN gain/bias broadcast tiles via GpSimd (engine is otherwise idle)
    ln_b = {}
    for nm in ("ln1g", "ln1b", "ln2g", "ln2b"):
        for l in range(LYR):
            t = const.tile([128, E], F32, tag=f"{nm}{l}")
            nc.gpsimd.partition_broadcast(t[:], rrow((nm, l)))
            ln_b[(nm, l)] = t

    ident = const.tile([128, 128], F32, tag="ident")
    make_identity(nc, ident[:])
    ones_row = const.tile([1, 128], F32, tag="ones_row")
    nc.vector.memset(ones_row[:], 1.0)
    ones_row_w = const.tile([1, 128], WDT, tag="ones_row_w")
    nc.vector.memset(ones_row_w[:], 1.0)
    eps_t = const.tile([128, 1], F32, tag="eps")
    nc.vector.memset(eps_t[:], EPS)

    def seed_bias(ps_ap, brow_ap, m):
        """PSUM <- bias row broadcast over m partitions (K=1 matmul)."""
        ones = ones_row if brow_ap.dtype == F32 else ones_row_w
        nc.tensor.matmul(ps_ap, ones[0:1, :m], brow_ap, start=True, stop=False)

    def layernorm(x_ap, rows_n, g_b, b_b, out_ap):
        st = work.tile([128, 6], F32, tag="bn_st")
        nc.vector.bn_stats(st[:rows_n], x_ap)
        mv = work.tile([128, 2], F32, tag="bn_mv")
        nc.vector.bn_aggr(mv[:rows_n], st[:rows_n])
        sd = work.tile([128, 1], F32, tag="bn_sd")
        nc.scalar.activation(sd[:rows_n], mv[:rows_n, 1:2], AF.Sqrt,
                             bias=eps_t[:rows_n])
        nc.vector.reciprocal(sd[:rows_n], sd[:rows_n])
        if g_b is None:  # final LN: gain/bias folded into W_proj on host
            nc.vector.tensor_scalar(out_ap, x_ap, mv[:rows_n, 0:1],
                                    sd[:rows_n], SUB, MULT)
            return
        t = work.tile([128, NPOS], F32, tag="ln_t")
        nc.vector.tensor_scalar(t[:rows_n], x_ap, mv[:rows_n, 0:1], sd[:rows_n],
                                SUB, MULT)
        nc.vector.tensor_mul(t[:rows_n], t[:rows_n], g_b[:rows_n])
        nc.vector.tensor_add(out_ap, t[:rows_n], b_b[:rows_n])

    # ---------------- embedding ----------------
    X_t, XT_t = {}, {}
    for p in range(2):  # position-major X
        ps = psw.tile([128, 512], F32, tag="qk")
        seed_bias(ps[:, :E], rrow("bemb_r"), 128)
        nc.tensor.matmul(ps[:, :E], xT_sb[:, p * 128:(p + 1) * 128],
                         Wemb_sb[:], start=False, stop=True)
        t = const.tile([128, NPOS], F32, tag=f"X{p}")
        nc.vector.tensor_copy(t[:], ps[:, :E])
        X_t[p] = t
    for k in range(2):  # channel-major XT
        ps = psw.tile([128, 512], F32, tag="qk")
        nc.tensor.matmul(ps[:, :NPOS], Wemb_sb[:, k * 128:(k + 1) * 128],
                         xT_sb[:], start=True, stop=True)
        t = const.tile([128, NPOS], WDT, tag=f"XT{k}")
        nc.vector.tensor_scalar_add(t[:], ps[:, :NPOS],
                                    fb("bemb_pp")[:, k:k + 1])
        XT_t[k] = t

    # ---------------- transformer layers ----------------
    for l in range(LYR):
        qlo, qhi = (0, NPOS) if l == 0 else (QLO2, NPOS)
        qw = qhi - qlo
        pos_chunks = ([(0, 0, 128), (1, 0, 128)] if l == 0
                      else [(1, 0, 128)])
        # (X-tile index, row offset within tile, nrows) for output positions

        # ---- K projection -> KT channel-major bf16 [128, 256] x2
        KT = {}
        for Jt in range(2):
            ps = psw.tile([128, 512], F32, tag="qk")
            for k in range(2):
                nc.tensor.matmul(
                    ps[:, :NPOS],
                    bf(("Wk", l, k))[:, Jt * 128:(Jt + 1) * 128],
                    XT_t[k][:], start=(k == 0), stop=(k == 1))
            t = work.tile([128, NPOS], BF16, tag=f"KT{Jt}")
            nc.vector.tensor_scalar_add(t[:], ps[:, :NPOS],
                                        fb(("bk", l))[:, Jt:Jt + 1])
            KT[Jt] = t

        # ---- V projection -> VO[pc] [128, 8, 64] bf16: per key chunk cp,
        # cols [0:32) = V channels, cols [32:64) = ones.  The combined
        # [V | 1] stationary makes ONE M=64 matmul per exp-region produce
        # both the A@V partial and the softmax row-sum.
        VO = {}
        for pc in range(2):
            ps = psw.tile([128, 512], F32, tag="qk")
            seed_bias(ps[:, :E], wrow(("bv", l)), 128)
            for k in range(2):
                nc.tensor.matmul(
                    ps[:, :E], XT_t[k][:, pc * 128:(pc + 1) * 128],
                    bf(("Wv", l, k))[:], start=False, stop=(k == 1))
            t = work.tile([128, 8, 64], BF16, tag=f"VO{pc}")
            nc.vector.tensor_copy(
                t[:, :, 0:32],
                ps[:, :E].rearrange("p (c d) -> p c d", d=32))
            nc.vector.memset(t[:, :, 32:64], 1.0)
            VO[pc] = t

        # ---- Q projection -> Qs_dbl [128, 2, 2, 4, qw] bf16
        # dims [part, h, dup, r, q]; rotation r written to both dups so a
        # row strip i can read 4 DOUBLED slots starting at (4-i)%4, which
        # enumerates q-chunks c = 4h+0..3 in ascending order.
        Qs_dbl = work.tile([128, 2, 2, 4, qw], BF16, tag=f"qsdbl{l}")
        for h in range(2):
            ps = psw.tile([128, 512], F32, tag="qk")
            for k in range(2):
                nc.tensor.matmul(
                    ps[:, :qw],
                    bf(("Wq", l, k))[:, h * 128:(h + 1) * 128],
                    XT_t[k][:, qlo:qhi], start=(k == 0), stop=(k == 1))
            tf = work.tile([128, NPOS], F32, tag="qtmp")
            nc.vector.tensor_scalar_add(tf[:, :qw], ps[:, :qw],
                                        fb(("bq", l))[:, h:h + 1])
            nc.vector.tensor_mul(Qs_dbl[:, h, 0, 0, :],
                                 tf[:, :qw], fb(("D", h))[:, qlo:qhi])
        for r in range(1, 4):
            for h in range(2):
                ps = psw.tile([128, 512], F32, tag="qk")
                nc.tensor.matmul(ps[:, :qw], bf(("Prot", r - 1))[:],
                                 Qs_dbl[:, h, 0, 0, :],
                                 start=True, stop=True)
                nc.vector.tensor_copy(Qs_dbl[:, h, 0, r, :], ps[:, :qw])
        for h in range(2):  # duplicate the 4 slots (wrap-around reads)
            nc.vector.tensor_copy(Qs_dbl[:, h, 1, :, :], Qs_dbl[:, h, 0, :, :])

        # ---- attention: ST -> exp -> [A@V | rowsum] accumulated in PSUM
        # OTR[64h+d,    c*qw+q] = attention out, q-chunk c=4h+c_local
        # OTR[64h+32+d, c*qw+q] = softmax denominator (identical over d)
        # zeroed by memset; all matmuls accumulate with start=False.
        OTR = psacc.tile([128, 1024], F32, tag="otr")
        nc.vector.memset(OTR[:], 0.0)
        qv = Qs_dbl[:].rearrange("p h u r q -> p (h u r) q")  # [128,16,qw]
        for J in range(2):          # key c'-quad
            for pc in range(2):     # key position chunk
                for i in range(4):  # key chunk cp = 4J+i
                    s0 = (4 - i) % 4
                    ats = []
                    for h in range(2):
                        # region [128,2,512]: one exp call; L1 = one h
                        # (4 tiles), L2 = both h (8 tiles)
                        if qw == NPOS or h == 0:
                            sc = scp.tile([128, 2, 512], F32, tag="sc")
                            at = atp.tile([128, 2, 512], BF16, tag="at")
                        for g in range(2 if qw == NPOS else 1):
                            # bank <- 2 (L1) / 4 (L2) c-ordered score tiles
                            bank = g if qw == NPOS else h
                            nslot = 2 if qw == NPOS else 4
                            nc.tensor.matmul(
                                sc[:, bank, :nslot * qw],
                                KT[J][32 * i:32 * (i + 1),
                                      pc * 128:(pc + 1) * 128],
                                qv[32 * i:32 * (i + 1),
                                   8 * h + s0 + g * 2:
                                   8 * h + s0 + g * 2 + nslot, :],
                                start=True, stop=True,
                                tile_position=(32 * i, 0))
                        if qw == NPOS:
                            nc.scalar.activation(at[:], sc[:], AF.Exp)
                            ats.append(at[:].rearrange("p b x -> p (b x)"))
                        elif h == 1:
                            nc.scalar.activation(at[:], sc[:], AF.Exp)
                            flat = at[:].rearrange("p b x -> p (b x)")
                            ats = [flat[:, 0:512], flat[:, 512:1024]]
                    cp = 4 * J + i
                    for h in range(2):
                        # psum-bank-sized chunks (out must not cross banks)
                        for o in range(0, 4 * qw, 512):
                            nc.tensor.matmul(
                                OTR[64 * h:64 * h + 64, o:o + 512],
                                VO[pc][:, cp, :], ats[h][:, o:o + 512],
                                start=False, stop=False,
                                skip_group_check=True,
                                tile_position=(0, 64 * h))

        # ---- normalize: OT = OT * (1/RS); redistribute [d,(c,q)] ->
        # [32c+d, q] channel-major via small SBUF->SBUF DMAs.
        otn = work.tile([128, 1024], F32, tag="otn")
        nc.vector.tensor_copy(otn[:, :4 * qw], OTR[:, :4 * qw])
        OT_sb = {}
        for h in range(2):
            ot128 = work.tile([128, NPOS], F32, tag=f"ot128{h}")
            rs128 = work.tile([128, NPOS], F32, tag=f"rs128{h}")
            for c in range(4):
                dma(out=ot128[32 * c:32 * c + 32, :qw],
                    in_=otn[64 * h:64 * h + 32, c * qw:(c + 1) * qw])
                dma(out=rs128[32 * c:32 * c + 32, :qw],
                    in_=otn[64 * h + 32:64 * h + 64, c * qw:(c + 1) * qw])
            rinv = work.tile([128, NPOS], F32, tag=f"rinv{h}")
            nc.vector.reciprocal_approx_fast(rinv[:, :qw], rs128[:, :qw])
            t = work.tile([128, NPOS], WDT, tag=f"OT{h}")
            nc.vector.tensor_tensor(t[:, :qw], ot128[:, :qw],
                                    rinv[:, :qw], MULT)
            OT_sb[h] = t

        # ---- O @ Wo + bo + residual -> LN1 -> xa
        xa = {}
        for ci, (xi, ro, nr) in enumerate(pos_chunks):
            ps = psw.tile([128, 512], F32, tag="qk")
            seed_bias(ps[:nr, :E], wrow(("bo", l)), nr)
            for h in range(2):
                nc.tensor.matmul(
                    ps[:nr, :E], OT_sb[h][:, ci * 128:ci * 128 + nr],
                    bf(("Wo", l, h))[:], start=False, stop=(h == 1))
            res = work.tile([128, NPOS], F32, tag=f"res{ci}")
            nc.vector.tensor_add(res[:nr], ps[:nr, :E],
                                 X_t[xi][ro:ro + nr, :])
            t = work.tile([128, NPOS], F32, tag=f"xa{ci}")
            layernorm(res[:nr], nr, ln_b[("ln1g", l)], ln_b[("ln1b", l)],
                      t[:nr])
            xa[ci] = t

        # ---- transpose xa -> xaT channel-major
        xaT = {}
        for k in range(2):
            t = work.tile([128, NPOS], WDT, tag=f"xaT{k}")
            for ci, (_, _, nr) in enumerate(pos_chunks):
                ps = psw.tile([128, 512], F32, tag="qk")
                nc.tensor.transpose(ps[:, :nr],
                                    xa[ci][:nr, k * 128:(k + 1) * 128],
                                    ident[:nr, :nr])
                nc.vector.tensor_copy(t[:, ci * 128:ci * 128 + nr],
                                      ps[:, :nr])
            xaT[k] = t

        # ---- FFN: H1T = relu(W1.T x + b1) channel-major bf16 [128, qw] x8
        H1T = {}
        for dk in range(8):
            ps = psw.tile([128, 512], F32, tag="qk")
            for k in range(2):
                nc.tensor.matmul(
                    ps[:, :qw],
                    bf(("W1", l, k))[:, dk * 128:(dk + 1) * 128],
                    xaT[k][:, :qw], start=(k == 0), stop=(k == 1))
            t = work.tile([128, NPOS], BF16, tag=f"H1T{dk}")
            nc.vector.tensor_scalar(t[:, :qw], ps[:, :qw],
                                    fb(("b1", l))[:, dk:dk + 1], 0.0,
                                    ADD, MAX)
            H1T[dk] = t

        # ---- FF = relu(H1 @ W2 + b2); X_next = LN2(xa + FF)
        newX = {}
        for ci, (_, _, nr) in enumerate(pos_chunks):
            ps = psw.tile([128, 512], F32, tag="qk")
            seed_bias(ps[:nr, :E], wrow(("b2", l)), nr)
            for dk in range(8):
                nc.tensor.matmul(
                    ps[:nr, :E], H1T[dk][:, ci * 128:ci * 128 + nr],
                    bf(("W2", l, dk))[:], start=False, stop=(dk == 7))
            t = work.tile([128, NPOS], F32, tag=f"ff{ci}")
            nc.vector.tensor_scalar_max(t[:nr], ps[:nr, :E], 0.0)
            res2 = work.tile([128, NPOS], F32, tag=f"res2{ci}")
            nc.vector.tensor_add(res2[:nr], t[:nr], xa[ci][:nr])
            xn = const.tile([128, NPOS], F32, tag=f"Xn{l}{ci}")
            layernorm(res2[:nr], nr, ln_b[("ln2g", l)], ln_b[("ln2b", l)],
                      xn[:nr])
            newX[ci] = xn

        if l == 0:
            X_t = {0: newX[0], 1: newX[1]}
            XT_t = {}
            for k in range(2):
                t = const.tile([128, NPOS], WDT, tag=f"X1T{k}")
                for ci in range(2):
                    ps = psw.tile([128, 512], F32, tag="qk")
                    nc.tensor.transpose(ps[:, :128],
                                        newX[ci][:, k * 128:(k + 1) * 128],
                                        ident[:])
                    nc.vector.tensor_copy(t[:, ci * 128:(ci + 1) * 128],
                                          ps[:, :128])
                XT_t[k] = t
        else:
            X2 = newX[0]  # [128, 256]

    # ---------------- final LN + projection ----------------
    xf = work.tile([128, NPOS], F32, tag="xf")
    layernorm(X2[:128], 128, None, None, xf[:128])
    xfT = {}
    for k in range(2):
        ps = psw.tile([128, 512], F32, tag="qk")
        nc.tensor.transpose(ps[:, :128], xf[:, k * 128:(k + 1) * 128],
                            ident[:])
        t = work.tile([128, 128], F32, tag=f"xfT{k}")
        nc.vector.tensor_copy(t[:], ps[:, :128])
        xfT[k] = t
    ps = psw.tile([128, 512], F32, tag="qk")
    nc.tensor.matmul(ps[:, 0:1], ones_row[0:1, :], rrow("bproj"),
                     start=True, stop=False)
    for k in range(2):
        nc.tensor.matmul(ps[:, 0:1], xfT[k][:], fb("Wp2")[:, k:k + 1],
                         start=False, stop=(k == 1))
    ot = work.tile([128, 1], F32, tag="outsb")
    nc.vector.tensor_copy(ot[:], ps[:, 0:1])
    # output = last 96 of the 128 computed positions
    nc.sync.dma_start(out=outs["out"][:], in_=ot[128 - PRED:, :])


# ======================= host side =======================

def _rot_matrices():
    """P_r[k, m] = 1 iff k = 32*((m//32 + r) % 4) + m % 32, r = 1..3."""
    mats = np.zeros((3, 128, 128), np.float32)
    for r in range(1, 4):
        for m in range(128):
            mats[r - 1, 32 * ((m // 32 + r) % 4) + m % 32, m] = 1.0
    return mats


def _make_in_maps(inputs):
    import ml_dtypes
    f = np.float32
    bh = ml_dtypes.bfloat16
    x_enc = np.asarray(inputs["x_enc"], f)
    td = np.asarray(inputs["time_diffs"], f)

    blob = np.zeros((128, BF_COLS), bh)
    rot = _rot_matrices()
    for r in range(3):
        blob[:, BF_OFF[("Prot", r)]:BF_OFF[("Prot", r)] + 128] = rot[r]
    for l in range(LYR):
        for nm in ("Wq", "Wk", "Wv"):
            w = np.asarray(inputs[nm], f)[l]
            for k in range(2):
                blob[:, BF_OFF[(nm, l, k)]:BF_OFF[(nm, l, k)] + E] = \
                    w[k * 128:(k + 1) * 128, :]
        wo = np.asarray(inputs["Wo"], f)[l]
        for h in range(2):
            blob[:, BF_OFF[("Wo", l, h)]:BF_OFF[("Wo", l, h)] + E] = \
                wo[h * 128:(h + 1) * 128, :]
        w1 = np.asarray(inputs["W1"], f)[l]
        for k in range(2):
            blob[:, BF_OFF[("W1", l, k)]:BF_OFF[("W1", l, k)] + DFF] = \
                w1[k * 128:(k + 1) * 128, :]
        w2 = np.asarray(inputs["W2"], f)[l]
        for dk in range(8):
            blob[:, BF_OFF[("W2", l, dk)]:BF_OFF[("W2", l, dk)] + E] = \
                w2[dk * 128:(dk + 1) * 128, :]

    fblob_base = np.zeros((128, F_COLS), f)
    for l in range(LYR):
        for nm, w in (("bq", 2), ("bk", 2), ("b1", 8)):
            arr = np.asarray(inputs[nm], f)[l].reshape(w, 128).T
            fblob_base[:, F_OFF[(nm, l)]:F_OFF[(nm, l)] + w] = arr
    fblob_base[:, F_OFF["bemb_pp"]:F_OFF["bemb_pp"] + 2] = \
        np.asarray(inputs["b_emb"], f).reshape(2, 128).T
    fblob_base[:, F_OFF["Wp2"]:F_OFF["Wp2"] + 2] = \
        (np.asarray(inputs["lnf_g"], f)
         * np.asarray(inputs["W_proj"], f)[:, 0]).reshape(2, 128).T

    rows = np.zeros((1, R_COLS), f)
    for nm, src in (("ln1g", "ln1_g"), ("ln1b", "ln1_b"),
                    ("ln2g", "ln2_g"), ("ln2b", "ln2_b")):
        for l in range(LYR):
            rows[0, R_OFF[(nm, l)]:R_OFF[(nm, l)] + E] = \
                np.asarray(inputs[src], f)[l]
    rows[0, R_OFF["bemb_r"]:R_OFF["bemb_r"] + E] = \
        np.asarray(inputs["b_emb"], f)
    # final LN gain/bias folded into the projection column:
    # (z*g + b) @ Wp0 + bp0 = z @ (g*Wp0) + (b@Wp0 + bp0)
    wp0 = np.asarray(inputs["W_proj"], f)[:, 0]
    lnfg = np.asarray(inputs["lnf_g"], f)
    lnfb = np.asarray(inputs["lnf_b"], f)
    rows[0, R_OFF["bproj"]] = (np.asarray(inputs["b_proj"], f)[0]
                               + float(lnfb @ wp0))

    wrows = np.zeros((1, WR_COLS), bh)
    for nm, src in (("bv", "bv"), ("bo", "bo"), ("b2", "b2")):
        for l in range(LYR):
            wrows[0, WR_OFF[(nm, l)]:WR_OFF[(nm, l)] + E] = \
                np.asarray(inputs[src], f)[l]

    maps = []
    for b in range(B):
        fblob = fblob_base.copy()
        # decay tiles: D[h][32j+d, q] = SCALE * exp(-td[q*8 + 4h+j] / F)
        dec = SCALE * np.exp(-td[b].reshape(NPOS, 8) / FACTOR)  # [q, chunk]
        for h in range(2):
            tile_ = np.repeat(dec[:, 4 * h:4 * h + 4].T, 32, axis=0)
            fblob[:, F_OFF[("D", h)]:F_OFF[("D", h)] + NPOS] = tile_
        xwa = np.zeros((D, 2 * NPOS), f)
        xwa[:, :NPOS] = x_enc[b, P0:P0 + NPOS, :].T
        xwa[:, NPOS:] = np.asarray(inputs["W_emb"], f)
        maps.append({
            "blob": blob, "fblob": np.ascontiguousarray(fblob),
            "rows": rows, "wrows": wrows,
            "xw": np.ascontiguousarray(xwa),
        })
    return maps


def _run(in_maps, check_with_sim=False, check_with_hw=True, **kw):
    from concourse.bass_test_utils import run_kernel

    n = len(in_maps)
    out_like = {"out": np.zeros((PRED, 1), np.float32)}
    res = run_kernel(
        lambda tc, outs, ins: chaos_kernel(tc, outs, ins),
        None,
        in_maps if n > 1 else in_maps[0],
        output_like=[out_like] * n if n > 1 else out_like,
        bass_type=tile.TileContext,
        num_cores=n,
        check_with_sim=check_with_sim,
        check_with_hw=check_with_hw,
        trace_sim=False,
        **kw,
    )
    return res


def kernel(**inputs):
    in_maps = _make_in_maps(inputs)
    res = _run(in_maps)
    out = np.stack(
        [list(res.results[b].values())[0].reshape(PRED) for b in range(B)])
    return out.astype(np.float32)


# revision 23
# speedup vs baseline: 1.7080x; 1.2233x over previous
"""Trainium2 Bass kernel for nn_ChaosTransformer_22333829939822.

Key mathematical reduction (verified against the reference):
the torch-style ``view(B, H, L, E//H)`` on a [B, L, E] tensor is a raw
row-major reshape, which makes head h attend only within the 256-position
block [h*256, (h+1)*256).  The output ``dec[:, -96:, 0]`` therefore depends
only on the last 256 positions of each batch.  Each core runs one batch's
[256, 256] residual-stream transformer; attention operates on the
[2048, 32] head-view of the 256x256 block.

Sharding: data-parallel over batch B across 4 of the 8 cores (one batch
per core, fully independent, no collectives).

v1 performance rework (from the 276us baseline trace):
- all weights ship in one bf16 blob + one small f32 blob (few large DMAs
  at fabric rate instead of ~100 descriptor-bound transfers).
- decay tiles D[h][32j+d, q] and all rearranged biases precomputed on host
  (the on-device build was 65k 4-byte DMA packets = 45us of dead time).
- LN gain/bias broadcast tiles built by the idle GpSimd engine.
- softmax row sums accumulated as M=32 ones-matmuls (same PE cost as M=1,
  yields the 32-row broadcast for free); 1/RS via reciprocal_approx_fast.
- score matmuls stream N=512 (2-4 score tiles per matmul) into 2-bank PSUM
  regions; ONE exp ACTIVATE per region amortizes ACT's 352-cycle fixed
  overhead (exp is the bottleneck engine: 1 elem/cyc/lane @ 1.2 GHz).
"""

import sys
import numpy as np

sys.path.insert(0, "/opt/trn_rl_repo")

import concourse.bass as bass
import concourse.tile as tile
from concourse import mybir
from concourse.masks import make_identity

F32 = mybir.dt.float32
BF16 = mybir.dt.bfloat16
WDT = BF16
ADD = mybir.AluOpType.add
SUB = mybir.AluOpType.subtract
MULT = mybir.AluOpType.mult
MAX = mybir.AluOpType.max
AF = mybir.ActivationFunctionType

B, L, D, E, DFF, LYR, PRED = 4, 2048, 7, 256, 1024, 2, 96
FACTOR = 5.0
SCALE = 1.0 / float(np.sqrt(FACTOR))
EPS = 1e-5
P0 = L - 256          # 1792: start of the last 256-position block
QLO2 = 128            # layer-2 computes query positions [128, 256)
NPOS = 256


# ---------------- blob layouts (host + device share these) ----------------

def _bf16_layout():
    """Column offsets into the [128, C] bf16 weight blob."""
    off = {}
    c = 0
    def put(key, w):
        nonlocal c
        off[key] = c
        c += w
    for r in range(3):
        put(("Prot", r), 128)
    for l in range(LYR):
        for k in range(2):
            put(("Wq", l, k), E)
            put(("Wk", l, k), E)
            put(("Wv", l, k), E)
        # WoR: rows [64h, 64h+32) col-block c hold Wo[l] rows 32*(4h+c);
        # consumed by K=32 matmuls against the [d,(c,q)] attention output.
        put(("WoR", l), 4 * E)
        for k in range(2):
            put(("W1", l, k), DFF)
        for dk in range(8):
            put(("W2", l, dk), E)
    return off, c


def _f32_layout():
    off = {}
    c = 0
    def put(key, w):
        nonlocal c
        off[key] = c
        c += w
    for h in range(2):
        put(("D", h), NPOS)
    for l in range(LYR):
        put(("bq", l), 2)
        put(("bk", l), 2)
        put(("b1", l), 8)
    put("bemb_pp", 2)
    put("Wp2", 2)
    return off, c


def _rows_layout():
    """f32 rows on partition 0: LN rows + f32 bias rows + bproj."""
    off = {}
    c = 0
    def put(key, w):
        nonlocal c
        off[key] = c
        c += w
    for nm in ("ln1g", "ln1b", "ln2g", "ln2b"):
        for l in range(LYR):
            put((nm, l), E)
    put("bemb_r", E)
    put("bproj", 1)
    return off, c


def _wrows_layout():
    """bf16 bias rows on partition 0 (seed rows for psum bias init)."""
    off = {}
    c = 0
    def put(key, w):
        nonlocal c
        off[key] = c
        c += w
    for nm in ("bv", "bo", "b2"):
        for l in range(LYR):
            put((nm, l), E)
    return off, c


BF_OFF, BF_COLS = _bf16_layout()
F_OFF, F_COLS = _f32_layout()
R_OFF, R_COLS = _rows_layout()
WR_OFF, WR_COLS = _wrows_layout()


def chaos_kernel(tc, outs, ins):
    import contextlib

    nc = tc.nc
    with contextlib.ExitStack() as ctx:
        _chaos_body(tc, nc, ctx, outs, ins)


def _chaos_body(tc, nc, ctx, outs, ins):
    const = ctx.enter_context(tc.tile_pool(name="const", bufs=1))
    work = ctx.enter_context(tc.tile_pool(name="work", bufs=2))
    atp = ctx.enter_context(tc.tile_pool(name="atp", bufs=12))
    psw = ctx.enter_context(tc.tile_pool(name="psw", bufs=2, space="PSUM"))
    scp = ctx.enter_context(tc.tile_pool(name="scp", bufs=2, space="PSUM"))
    psacc = ctx.enter_context(tc.tile_pool(name="psacc", bufs=1, space="PSUM"))

    dma = nc.sync.dma_start

    # ---------------- constant loads (few big DMAs) ----------------
    fblob = const.tile([128, F_COLS], F32, tag="fblob")
    dma(out=fblob[:], in_=ins["fblob"][:])
    rows = const.tile([1, R_COLS], F32, tag="rows")
    dma(out=rows[:], in_=ins["rows"][:])
    wrows = const.tile([1, WR_COLS], BF16, tag="wrows")
    dma(out=wrows[:], in_=ins["wrows"][:])
    xw = const.tile([D, 2 * NPOS], F32, tag="xw")
    dma(out=xw[:], in_=ins["xw"][:])
    blob = const.tile([128, BF_COLS], BF16, tag="blob")
    # split: layer-0 weights (+Prot) first so compute can start early
    split = BF_OFF[("Wq", 1, 0)]
    dma(out=blob[:, :split], in_=ins["blob"][:, :split])
    dma(out=blob[:, split:], in_=ins["blob"][:, split:])
    xT_sb = xw[:, :NPOS]
    Wemb_sb = xw[:, NPOS:]

    def bf(key):
        w = {"Prot": 128, "W1": DFF, "WoR": 4 * E}.get(key[0], E)
        return blob[:, BF_OFF[key]:BF_OFF[key] + w]

    def fb(key):
        w = {"D": NPOS, "b1": 8}.get(key[0] if isinstance(key, tuple) else key, 2)
        return fblob[:, F_OFF[key]:F_OFF[key] + w]

    def rrow(key):
        w = 1 if key == "bproj" else E
        return rows[0:1, R_OFF[key]:R_OFF[key] + w]

    def wrow(key):
        return wrows[0:1, WR_OFF[key]:WR_OFF[key] + E]

    # LN gain/bias broadcast tiles: ONE GpSimd broadcast for all 8 rows
    # (the rows are contiguous at the start of the rows blob)
    ln_w = 4 * LYR * E
    lnall = const.tile([128, ln_w], F32, tag="lnall")
    nc.gpsimd.partition_broadcast(lnall[:], rows[0:1, :ln_w])
    ln_b = {}
    for ni, nm in enumerate(("ln1g", "ln1b", "ln2g", "ln2b")):
        for l in range(LYR):
            o = R_OFF[(nm, l)]
            ln_b[(nm, l)] = lnall[:, o:o + E]

    ident = const.tile([128, 128], F32, tag="ident")
    make_identity(nc, ident[:])
    ones_row = const.tile([1, 128], F32, tag="ones_row")
    nc.vector.memset(ones_row[:], 1.0)
    ones_row_w = const.tile([1, 128], WDT, tag="ones_row_w")
    nc.vector.memset(ones_row_w[:], 1.0)
    eps_t = const.tile([128, 1], F32, tag="eps")
    nc.vector.memset(eps_t[:], EPS)

    def seed_bias(ps_ap, brow_ap, m):
        """PSUM <- bias row broadcast over m partitions (K=1 matmul)."""
        ones = ones_row if brow_ap.dtype == F32 else ones_row_w
        nc.tensor.matmul(ps_ap, ones[0:1, :m], brow_ap, start=True, stop=False)

    def layernorm(x_ap, rows_n, g_b, b_b, out_ap):
        st = work.tile([128, 6], F32, tag="bn_st")
        nc.vector.bn_stats(st[:rows_n], x_ap)
        mv = work.tile([128, 2], F32, tag="bn_mv")
        nc.vector.bn_aggr(mv[:rows_n], st[:rows_n])
        sd = work.tile([128, 1], F32, tag="bn_sd")
        nc.scalar.activation(sd[:rows_n], mv[:rows_n, 1:2], AF.Sqrt,
                             bias=eps_t[:rows_n])
        nc.vector.reciprocal(sd[:rows_n], sd[:rows_n])
        if g_b is None:  # final LN: gain/bias folded into W_proj on host
            nc.vector.tensor_scalar(out_ap, x_ap, mv[:rows_n, 0:1],
                                    sd[:rows_n], SUB, MULT)
            return
        t = work.tile([128, NPOS], F32, tag="ln_t")
        nc.vector.tensor_scalar(t[:rows_n], x_ap, mv[:rows_n, 0:1], sd[:rows_n],
                                SUB, MULT)
        nc.vector.tensor_mul(t[:rows_n], t[:rows_n], g_b[:rows_n])
        nc.vector.tensor_add(out_ap, t[:rows_n], b_b[:rows_n])

    # ---------------- embedding ----------------
    X_t, XT_t = {}, {}
    for p in range(2):  # position-major X
        ps = psw.tile([128, 512], F32, tag="qk")
        seed_bias(ps[:, :E], rrow("bemb_r"), 128)
        nc.tensor.matmul(ps[:, :E], xT_sb[:, p * 128:(p + 1) * 128],
                         Wemb_sb[:], start=False, stop=True)
        t = const.tile([128, NPOS], F32, tag=f"X{p}")
        nc.vector.tensor_copy(t[:], ps[:, :E])
        X_t[p] = t
    for k in range(2):  # channel-major XT
        ps = psw.tile([128, 512], F32, tag="qk")
        nc.tensor.matmul(ps[:, :NPOS], Wemb_sb[:, k * 128:(k + 1) * 128],
                         xT_sb[:], start=True, stop=True)
        t = const.tile([128, NPOS], WDT, tag=f"XT{k}")
        nc.vector.tensor_scalar_add(t[:], ps[:, :NPOS],
                                    fb("bemb_pp")[:, k:k + 1])
        XT_t[k] = t

    # ---------------- transformer layers ----------------
    for l in range(LYR):
        qlo, qhi = (0, NPOS) if l == 0 else (QLO2, NPOS)
        qw = qhi - qlo
        pos_chunks = ([(0, 0, 128), (1, 0, 128)] if l == 0
                      else [(1, 0, 128)])
        # (X-tile index, row offset within tile, nrows) for output positions

        # ---- K projection -> KT channel-major bf16 [128, 256] x2
        KT = {}
        for Jt in range(2):
            ps = psw.tile([128, 512], F32, tag="qk")
            for k in range(2):
                nc.tensor.matmul(
                    ps[:, :NPOS],
                    bf(("Wk", l, k))[:, Jt * 128:(Jt + 1) * 128],
                    XT_t[k][:], start=(k == 0), stop=(k == 1))
            t = work.tile([128, NPOS], BF16, tag=f"KT{Jt}")
            nc.vector.tensor_scalar_add(t[:], ps[:, :NPOS],
                                        fb(("bk", l))[:, Jt:Jt + 1])
            KT[Jt] = t

        # ---- V projection -> VO[pc] [128, 8, 64] bf16: per key chunk cp,
        # cols [0:32) = V channels, cols [32:64) = ones.  The combined
        # [V | 1] stationary makes ONE M=64 matmul per exp-region produce
        # both the A@V partial and the softmax row-sum.
        VO = {}
        for pc in range(2):
            ps = psw.tile([128, 512], F32, tag="qk")
            seed_bias(ps[:, :E], wrow(("bv", l)), 128)
            for k in range(2):
                nc.tensor.matmul(
                    ps[:, :E], XT_t[k][:, pc * 128:(pc + 1) * 128],
                    bf(("Wv", l, k))[:], start=False, stop=(k == 1))
            t = work.tile([128, 8, 64], BF16, tag=f"VO{pc}")
            nc.vector.tensor_copy(
                t[:, :, 0:32],
                ps[:, :E].rearrange("p (c d) -> p c d", d=32))
            nc.vector.memset(t[:, :, 32:64], 1.0)
            VO[pc] = t

        # ---- Q projection -> Qs_dbl [128, 2, 2, 4, qw] bf16
        # dims [part, h, dup, r, q]; rotation r written to both dups so a
        # row strip i can read 4 DOUBLED slots starting at (4-i)%4, which
        # enumerates q-chunks c = 4h+0..3 in ascending order.
        Qs_dbl = work.tile([128, 2, 2, 4, qw], BF16, tag=f"qsdbl{l}")
        for h in range(2):
            ps = psw.tile([128, 512], F32, tag="qk")
            for k in range(2):
                nc.tensor.matmul(
                    ps[:, :qw],
                    bf(("Wq", l, k))[:, h * 128:(h + 1) * 128],
                    XT_t[k][:, qlo:qhi], start=(k == 0), stop=(k == 1))
            tf = work.tile([128, NPOS], F32, tag="qtmp")
            nc.vector.tensor_scalar_add(tf[:, :qw], ps[:, :qw],
                                        fb(("bq", l))[:, h:h + 1])
            nc.vector.tensor_mul(Qs_dbl[:, h, 0, 0, :],
                                 tf[:, :qw], fb(("D", h))[:, qlo:qhi])
        for r in range(1, 4):
            for h in range(2):
                ps = psw.tile([128, 512], F32, tag="qk")
                nc.tensor.matmul(ps[:, :qw], bf(("Prot", r - 1))[:],
                                 Qs_dbl[:, h, 0, 0, :],
                                 start=True, stop=True)
                nc.vector.tensor_copy(Qs_dbl[:, h, 0, r, :], ps[:, :qw])
        for h in range(2):  # duplicate the 4 slots (wrap-around reads)
            nc.vector.tensor_copy(Qs_dbl[:, h, 1, :, :], Qs_dbl[:, h, 0, :, :])

        # ---- attention: ST -> exp -> [A@V | rowsum] accumulated in PSUM
        # OTR[64h+d,    c*qw+q] = attention out, q-chunk c=4h+c_local
        # OTR[64h+32+d, c*qw+q] = softmax denominator (identical over d)
        # zeroed by memset; all matmuls accumulate with start=False.
        OTR = psacc.tile([128, 1024], F32, tag="otr")
        nc.vector.memset(OTR[:], 0.0)
        qv = Qs_dbl[:].rearrange("p h u r q -> p (h u r) q")  # [128,16,qw]
        ng = 4 * qw // 512       # score banks per (i,h): 2 for L1, 1 for L2
        nslot = 512 // qw        # c-slots per matmul: 2 for L1, 4 for L2
        for J in range(2):          # key c'-quad
            for pc in range(2):     # key position chunk
                # QK: consecutive matmuls walk strips i=2a, 2a+1, ... so
                # the K=32 row-tiles stream concurrently in the PE array.
                AT = {}
                for h in range(2):
                    for g in range(ng):
                        for a in range(2):
                            sc = scp.tile([128, 2, 512], F32, tag="sc")
                            at = atp.tile([128, 2, 512], BF16, tag="at")
                            for b in range(2):
                                i = 2 * a + b
                                s0 = (4 - i) % 4
                                nc.tensor.matmul(
                                    sc[:, b, :nslot * qw],
                                    KT[J][32 * i:32 * (i + 1),
                                          pc * 128:(pc + 1) * 128],
                                    qv[32 * i:32 * (i + 1),
                                       8 * h + s0 + g * nslot:
                                       8 * h + s0 + (g + 1) * nslot, :],
                                    start=True, stop=True,
                                    tile_position=(32 * i, 0))
                            nc.scalar.activation(at[:], sc[:], AF.Exp)
                            AT[(h, g, a)] = at
                # AV + rowsum: one M=64 matmul per psum bank of scores
                for i in range(4):  # key chunk cp = 4J+i
                    cp = 4 * J + i
                    for h in range(2):
                        for o in range(ng):
                            nc.tensor.matmul(
                                OTR[64 * h:64 * h + 64,
                                    o * 512:o * 512 + 512],
                                VO[pc][:, cp, :],
                                AT[(h, o, i // 2)][:, i % 2, :],
                                start=False, stop=False,
                                skip_group_check=True,
                                tile_position=(0, 64 * h))

        # ---- normalize in the [d, (c,q)] layout (no redistribution):
        # copy RS rows down 32 partitions, recip in place, multiply the
        # PSUM accumulator directly -> bf16 otn consumed by WoR matmuls.
        rsal = work.tile([128, 1024], F32, tag="rsal")
        rinv = work.tile([128, 1024], F32, tag="rinv")
        otn = work.tile([128, 1024], WDT, tag="otn")
        for h in range(2):
            nc.vector.tensor_copy(rsal[64 * h:64 * h + 32, :4 * qw],
                                  OTR[64 * h + 32:64 * h + 64, :4 * qw])
            nc.vector.reciprocal_approx_fast(
                rinv[64 * h:64 * h + 32, :4 * qw],
                rsal[64 * h:64 * h + 32, :4 * qw])
            nc.vector.tensor_tensor(otn[64 * h:64 * h + 32, :4 * qw],
                                    OTR[64 * h:64 * h + 32, :4 * qw],
                                    rinv[64 * h:64 * h + 32, :4 * qw], MULT)

        # ---- O @ Wo + bo + residual -> LN1 -> xa
        xa = {}
        for ci, (xi, ro, nr) in enumerate(pos_chunks):
            ps = psw.tile([128, 512], F32, tag="qk")
            seed_bias(ps[:nr, :E], wrow(("bo", l)), nr)
            for h in range(2):
                for c in range(4):
                    nc.tensor.matmul(
                        ps[:nr, :E],
                        otn[64 * h:64 * h + 32,
                            c * qw + ci * 128:c * qw + ci * 128 + nr],
                        bf(("WoR", l))[64 * h:64 * h + 32,
                                       c * E:(c + 1) * E],
                        start=False, stop=(h == 1 and c == 3),
                        tile_position=(64 * h, 0))
            res = work.tile([128, NPOS], F32, tag=f"res{ci}")
            nc.vector.tensor_add(res[:nr], ps[:nr, :E],
                                 X_t[xi][ro:ro + nr, :])
            t = work.tile([128, NPOS], F32, tag=f"xa{ci}")
            layernorm(res[:nr], nr, ln_b[("ln1g", l)], ln_b[("ln1b", l)],
                      t[:nr])
            xa[ci] = t

        # ---- transpose xa -> xaT channel-major
        xaT = {}
        for k in range(2):
            t = work.tile([128, NPOS], WDT, tag=f"xaT{k}")
            for ci, (_, _, nr) in enumerate(pos_chunks):
                ps = psw.tile([128, 512], F32, tag="qk")
                nc.tensor.transpose(ps[:, :nr],
                                    xa[ci][:nr, k * 128:(k + 1) * 128],
                                    ident[:nr, :nr])
                nc.vector.tensor_copy(t[:, ci * 128:ci * 128 + nr],
                                      ps[:, :nr])
            xaT[k] = t

        # ---- FFN: H1T = relu(W1.T x + b1) channel-major bf16 [128, qw] x8
        H1T = {}
        for dk in range(8):
            ps = psw.tile([128, 512], F32, tag="qk")
            for k in range(2):
                nc.tensor.matmul(
                    ps[:, :qw],
                    bf(("W1", l, k))[:, dk * 128:(dk + 1) * 128],
                    xaT[k][:, :qw], start=(k == 0), stop=(k == 1))
            t = work.tile([128, NPOS], BF16, tag=f"H1T{dk}")
            nc.vector.tensor_scalar(t[:, :qw], ps[:, :qw],
                                    fb(("b1", l))[:, dk:dk + 1], 0.0,
                                    ADD, MAX)
            H1T[dk] = t

        # ---- FF = relu(H1 @ W2 + b2); X_next = LN2(xa + FF)
        newX = {}
        for ci, (_, _, nr) in enumerate(pos_chunks):
            ps = psw.tile([128, 512], F32, tag="qk")
            seed_bias(ps[:nr, :E], wrow(("b2", l)), nr)
            for dk in range(8):
                nc.tensor.matmul(
                    ps[:nr, :E], H1T[dk][:, ci * 128:ci * 128 + nr],
                    bf(("W2", l, dk))[:], start=False, stop=(dk == 7))
            t = work.tile([128, NPOS], F32, tag=f"ff{ci}")
            nc.vector.tensor_scalar_max(t[:nr], ps[:nr, :E], 0.0)
            res2 = work.tile([128, NPOS], F32, tag=f"res2{ci}")
            nc.vector.tensor_add(res2[:nr], t[:nr], xa[ci][:nr])
            xn = const.tile([128, NPOS], F32, tag=f"Xn{l}{ci}")
            layernorm(res2[:nr], nr, ln_b[("ln2g", l)], ln_b[("ln2b", l)],
                      xn[:nr])
            newX[ci] = xn

        if l == 0:
            X_t = {0: newX[0], 1: newX[1]}
            XT_t = {}
            for k in range(2):
                t = const.tile([128, NPOS], WDT, tag=f"X1T{k}")
                for ci in range(2):
                    ps = psw.tile([128, 512], F32, tag="qk")
                    nc.tensor.transpose(ps[:, :128],
                                        newX[ci][:, k * 128:(k + 1) * 128],
                                        ident[:])
                    nc.vector.tensor_copy(t[:, ci * 128:(ci + 1) * 128],
                                          ps[:, :128])
                XT_t[k] = t
        else:
            X2 = newX[0]  # [128, 256]

    # ---------------- final LN + projection ----------------
    xf = work.tile([128, NPOS], F32, tag="xf")
    layernorm(X2[:128], 128, None, None, xf[:128])
    xfT = {}
    for k in range(2):
        ps = psw.tile([128, 512], F32, tag="qk")
        nc.tensor.transpose(ps[:, :128], xf[:, k * 128:(k + 1) * 128],
                            ident[:])
        t = work.tile([128, 128], F32, tag=f"xfT{k}")
        nc.vector.tensor_copy(t[:], ps[:, :128])
        xfT[k] = t
    ps = psw.tile([128, 512], F32, tag="qk")
    nc.tensor.matmul(ps[:, 0:1], ones_row[0:1, :], rrow("bproj"),
                     start=True, stop=False)
    for k in range(2):
        nc.tensor.matmul(ps[:, 0:1], xfT[k][:], fb("Wp2")[:, k:k + 1],
                         start=False, stop=(k == 1))
    ot = work.tile([128, 1], F32, tag="outsb")
    nc.vector.tensor_copy(ot[:], ps[:, 0:1])
    # output = last 96 of the 128 computed positions
    nc.sync.dma_start(out=outs["out"][:], in_=ot[128 - PRED:, :])


# ======================= host side =======================

def _rot_matrices():
    """P_r[k, m] = 1 iff k = 32*((m//32 + r) % 4) + m % 32, r = 1..3."""
    mats = np.zeros((3, 128, 128), np.float32)
    for r in range(1, 4):
        for m in range(128):
            mats[r - 1, 32 * ((m // 32 + r) % 4) + m % 32, m] = 1.0
    return mats


def _make_in_maps(inputs):
    import ml_dtypes
    f = np.float32
    bh = ml_dtypes.bfloat16
    x_enc = np.asarray(inputs["x_enc"], f)
    td = np.asarray(inputs["time_diffs"], f)

    blob = np.zeros((128, BF_COLS), bh)
    rot = _rot_matrices()
    for r in range(3):
        blob[:, BF_OFF[("Prot", r)]:BF_OFF[("Prot", r)] + 128] = rot[r]
    for l in range(LYR):
        for nm in ("Wq", "Wk", "Wv"):
            w = np.asarray(inputs[nm], f)[l]
            for k in range(2):
                blob[:, BF_OFF[(nm, l, k)]:BF_OFF[(nm, l, k)] + E] = \
                    w[k * 128:(k + 1) * 128, :]
        wo = np.asarray(inputs["Wo"], f)[l]
        wor = np.zeros((128, 4 * E), f)
        for h in range(2):
            for c in range(4):
                wor[64 * h:64 * h + 32, c * E:(c + 1) * E] = \
                    wo[32 * (4 * h + c):32 * (4 * h + c) + 32, :]
        blob[:, BF_OFF[("WoR", l)]:BF_OFF[("WoR", l)] + 4 * E] = wor
        w1 = np.asarray(inputs["W1"], f)[l]
        for k in range(2):
            blob[:, BF_OFF[("W1", l, k)]:BF_OFF[("W1", l, k)] + DFF] = \
                w1[k * 128:(k + 1) * 128, :]
        w2 = np.asarray(inputs["W2"], f)[l]
        for dk in range(8):
            blob[:, BF_OFF[("W2", l, dk)]:BF_OFF[("W2", l, dk)] + E] = \
                w2[dk * 128:(dk + 1) * 128, :]

    fblob_base = np.zeros((128, F_COLS), f)
    for l in range(LYR):
        for nm, w in (("bq", 2), ("bk", 2), ("b1", 8)):
            arr = np.asarray(inputs[nm], f)[l].reshape(w, 128).T
            fblob_base[:, F_OFF[(nm, l)]:F_OFF[(nm, l)] + w] = arr
    fblob_base[:, F_OFF["bemb_pp"]:F_OFF["bemb_pp"] + 2] = \
        np.asarray(inputs["b_emb"], f).reshape(2, 128).T
    fblob_base[:, F_OFF["Wp2"]:F_OFF["Wp2"] + 2] = \
        (np.asarray(inputs["lnf_g"], f)
         * np.asarray(inputs["W_proj"], f)[:, 0]).reshape(2, 128).T

    rows = np.zeros((1, R_COLS), f)
    for nm, src in (("ln1g", "ln1_g"), ("ln1b", "ln1_b"),
                    ("ln2g", "ln2_g"), ("ln2b", "ln2_b")):
        for l in range(LYR):
            rows[0, R_OFF[(nm, l)]:R_OFF[(nm, l)] + E] = \
                np.asarray(inputs[src], f)[l]
    rows[0, R_OFF["bemb_r"]:R_OFF["bemb_r"] + E] = \
        np.asarray(inputs["b_emb"], f)
    # final LN gain/bias folded into the projection column:
    # (z*g + b) @ Wp0 + bp0 = z @ (g*Wp0) + (b@Wp0 + bp0)
    wp0 = np.asarray(inputs["W_proj"], f)[:, 0]
    lnfg = np.asarray(inputs["lnf_g"], f)
    lnfb = np.asarray(inputs["lnf_b"], f)
    rows[0, R_OFF["bproj"]] = (np.asarray(inputs["b_proj"], f)[0]
                               + float(lnfb @ wp0))

    wrows = np.zeros((1, WR_COLS), bh)
    for nm, src in (("bv", "bv"), ("bo", "bo"), ("b2", "b2")):
        for l in range(LYR):
            wrows[0, WR_OFF[(nm, l)]:WR_OFF[(nm, l)] + E] = \
                np.asarray(inputs[src], f)[l]

    maps = []
    for b in range(B):
        fblob = fblob_base.copy()
        # decay tiles: D[h][32j+d, q] = SCALE * exp(-td[q*8 + 4h+j] / F)
        dec = SCALE * np.exp(-td[b].reshape(NPOS, 8) / FACTOR)  # [q, chunk]
        for h in range(2):
            tile_ = np.repeat(dec[:, 4 * h:4 * h + 4].T, 32, axis=0)
            fblob[:, F_OFF[("D", h)]:F_OFF[("D", h)] + NPOS] = tile_
        xwa = np.zeros((D, 2 * NPOS), f)
        xwa[:, :NPOS] = x_enc[b, P0:P0 + NPOS, :].T
        xwa[:, NPOS:] = np.asarray(inputs["W_emb"], f)
        maps.append({
            "blob": blob, "fblob": np.ascontiguousarray(fblob),
            "rows": rows, "wrows": wrows,
            "xw": np.ascontiguousarray(xwa),
        })
    return maps


def _run(in_maps, check_with_sim=False, check_with_hw=True, **kw):
    from concourse.bass_test_utils import run_kernel

    n = len(in_maps)
    out_like = {"out": np.zeros((PRED, 1), np.float32)}
    res = run_kernel(
        lambda tc, outs, ins: chaos_kernel(tc, outs, ins),
        None,
        in_maps if n > 1 else in_maps[0],
        output_like=[out_like] * n if n > 1 else out_like,
        bass_type=tile.TileContext,
        num_cores=n,
        check_with_sim=check_with_sim,
        check_with_hw=check_with_hw,
        trace_sim=False,
        **kw,
    )
    return res


def kernel(**inputs):
    in_maps = _make_in_maps(inputs)
    res = _run(in_maps)
    out = np.stack(
        [list(res.results[b].values())[0].reshape(PRED) for b in range(B)])
    return out.astype(np.float32)


# revision 35
# speedup vs baseline: 1.7774x; 1.0406x over previous
"""Trainium2 Bass kernel for nn_ChaosTransformer_22333829939822.

Key mathematical reduction (verified against the reference):
the torch-style ``view(B, H, L, E//H)`` on a [B, L, E] tensor is a raw
row-major reshape, which makes head h attend only within the 256-position
block [h*256, (h+1)*256).  The output ``dec[:, -96:, 0]`` therefore depends
only on the last 256 positions of each batch.  Each core runs one batch's
[256, 256] residual-stream transformer; attention operates on the
[2048, 32] head-view of the 256x256 block.

Sharding: data-parallel over batch B across 4 of the 8 cores (one batch
per core, fully independent, no collectives).

v1 performance rework (from the 276us baseline trace):
- all weights ship in one bf16 blob + one small f32 blob (few large DMAs
  at fabric rate instead of ~100 descriptor-bound transfers).
- decay tiles D[h][32j+d, q] and all rearranged biases precomputed on host
  (the on-device build was 65k 4-byte DMA packets = 45us of dead time).
- LN gain/bias broadcast tiles built by the idle GpSimd engine.
- softmax row sums accumulated as M=32 ones-matmuls (same PE cost as M=1,
  yields the 32-row broadcast for free); 1/RS via reciprocal_approx_fast.
- score matmuls stream N=512 (2-4 score tiles per matmul) into 2-bank PSUM
  regions; ONE exp ACTIVATE per region amortizes ACT's 352-cycle fixed
  overhead (exp is the bottleneck engine: 1 elem/cyc/lane @ 1.2 GHz).
"""

import sys
import numpy as np

sys.path.insert(0, "/opt/trn_rl_repo")

import concourse.bass as bass
import concourse.tile as tile
from concourse import mybir
from concourse.masks import make_identity

F32 = mybir.dt.float32
BF16 = mybir.dt.bfloat16
WDT = BF16
ADD = mybir.AluOpType.add
SUB = mybir.AluOpType.subtract
MULT = mybir.AluOpType.mult
MAX = mybir.AluOpType.max
AF = mybir.ActivationFunctionType

B, L, D, E, DFF, LYR, PRED = 4, 2048, 7, 256, 1024, 2, 96
FACTOR = 5.0
SCALE = 1.0 / float(np.sqrt(FACTOR))
EPS = 1e-5
P0 = L - 256          # 1792: start of the last 256-position block
QLO2 = 160            # layer-2 computes exactly the output positions
NPOS = 256


# ---------------- blob layouts (host + device share these) ----------------

def _bf16_layout():
    """Column offsets into the [128, C] bf16 weight blob."""
    off = {}
    c = 0
    def put(key, w):
        nonlocal c
        off[key] = c
        c += w
    for r in range(3):
        put(("Prot", r), 128)
    for l in range(LYR):
        for k in range(2):
            put(("Wq", l, k), E)
            put(("Wk", l, k), E)
            put(("Wv", l, k), E)
        # WoR: rows [64h, 64h+32) col-block c hold Wo[l] rows 32*(4h+c);
        # consumed by K=32 matmuls against the [d,(c,q)] attention output.
        put(("WoR", l), 4 * E)
        for k in range(2):
            put(("W1", l, k), DFF)
        for dk in range(8):
            put(("W2", l, dk), E)
    return off, c


def _f32_layout():
    off = {}
    c = 0
    def put(key, w):
        nonlocal c
        off[key] = c
        c += w
    for h in range(2):
        put(("D", h), NPOS)
    for l in range(LYR):
        put(("bq", l), 2)
        put(("bk", l), 2)
        put(("b1", l), 8)
    put("bemb_pp", 2)
    put("Wp2", 2)
    return off, c


def _rows_layout():
    """f32 rows on partition 0: LN rows + f32 bias rows + bproj."""
    off = {}
    c = 0
    def put(key, w):
        nonlocal c
        off[key] = c
        c += w
    for nm in ("ln1g", "ln1b", "ln2g", "ln2b"):
        for l in range(LYR):
            put((nm, l), E)
    put("bemb_r", E)
    put("bproj", 1)
    return off, c


def _wrows_layout():
    """bf16 bias rows on partition 0 (seed rows for psum bias init)."""
    off = {}
    c = 0
    def put(key, w):
        nonlocal c
        off[key] = c
        c += w
    for nm in ("bv", "bo", "b2"):
        for l in range(LYR):
            put((nm, l), E)
    return off, c


BF_OFF, BF_COLS = _bf16_layout()
F_OFF, F_COLS = _f32_layout()
R_OFF, R_COLS = _rows_layout()
WR_OFF, WR_COLS = _wrows_layout()


def chaos_kernel(tc, outs, ins):
    import contextlib

    nc = tc.nc
    with contextlib.ExitStack() as ctx:
        _chaos_body(tc, nc, ctx, outs, ins)


def _chaos_body(tc, nc, ctx, outs, ins):
    const = ctx.enter_context(tc.tile_pool(name="const", bufs=1))
    work = ctx.enter_context(tc.tile_pool(name="work", bufs=2))
    atp = ctx.enter_context(tc.tile_pool(name="atp", bufs=12))
    psw = ctx.enter_context(tc.tile_pool(name="psw", bufs=2, space="PSUM"))
    scp = ctx.enter_context(tc.tile_pool(name="scp", bufs=2, space="PSUM"))
    psacc = ctx.enter_context(tc.tile_pool(name="psacc", bufs=1, space="PSUM"))

    dma = nc.sync.dma_start

    # ---------------- constant loads (few big DMAs) ----------------
    fblob = const.tile([128, F_COLS], F32, tag="fblob")
    dma(out=fblob[:], in_=ins["fblob"][:])
    rows = const.tile([1, R_COLS], F32, tag="rows")
    dma(out=rows[:], in_=ins["rows"][:])
    wrows = const.tile([1, WR_COLS], BF16, tag="wrows")
    dma(out=wrows[:], in_=ins["wrows"][:])
    xw = const.tile([D, 2 * NPOS], F32, tag="xw")
    dma(out=xw[:], in_=ins["xw"][:])
    blob = const.tile([128, BF_COLS], BF16, tag="blob")
    # split: Prot + layer-0 QKV first so attention can start early
    s1 = BF_OFF[("WoR", 0)]
    s2 = BF_OFF[("Wq", 1, 0)]
    dma(out=blob[:, :s1], in_=ins["blob"][:, :s1])
    dma(out=blob[:, s1:s2], in_=ins["blob"][:, s1:s2])
    dma(out=blob[:, s2:], in_=ins["blob"][:, s2:])
    xT_sb = xw[:, :NPOS]
    Wemb_sb = xw[:, NPOS:]

    def bf(key):
        w = {"Prot": 128, "W1": DFF, "WoR": 4 * E}.get(key[0], E)
        return blob[:, BF_OFF[key]:BF_OFF[key] + w]

    def fb(key):
        w = {"D": NPOS, "b1": 8}.get(key[0] if isinstance(key, tuple) else key, 2)
        return fblob[:, F_OFF[key]:F_OFF[key] + w]

    def rrow(key):
        w = 1 if key == "bproj" else E
        return rows[0:1, R_OFF[key]:R_OFF[key] + w]

    def wrow(key):
        return wrows[0:1, WR_OFF[key]:WR_OFF[key] + E]

    # LN gain/bias broadcast tiles: ONE GpSimd broadcast for all 8 rows
    # (the rows are contiguous at the start of the rows blob)
    ln_w = 4 * LYR * E
    lnall = const.tile([128, ln_w], F32, tag="lnall")
    nc.gpsimd.partition_broadcast(lnall[:], rows[0:1, :ln_w])
    ln_b = {}
    for ni, nm in enumerate(("ln1g", "ln1b", "ln2g", "ln2b")):
        for l in range(LYR):
            o = R_OFF[(nm, l)]
            ln_b[(nm, l)] = lnall[:, o:o + E]

    ident = const.tile([128, 128], F32, tag="ident")
    make_identity(nc, ident[:])
    ones_row = const.tile([1, 128], F32, tag="ones_row")
    nc.vector.memset(ones_row[:], 1.0)
    ones_row_w = const.tile([1, 128], WDT, tag="ones_row_w")
    nc.vector.memset(ones_row_w[:], 1.0)
    eps_t = const.tile([128, 1], F32, tag="eps")
    nc.vector.memset(eps_t[:], EPS)

    def seed_bias(ps_ap, brow_ap, m):
        """PSUM <- bias row broadcast over m partitions (K=1 matmul)."""
        ones = ones_row if brow_ap.dtype == F32 else ones_row_w
        nc.tensor.matmul(ps_ap, ones[0:1, :m], brow_ap, start=True, stop=False)

    def layernorm(x_ap, rows_n, g_b, b_b, out_ap):
        st = work.tile([128, 6], F32, tag="bn_st")
        nc.vector.bn_stats(st[:rows_n], x_ap)
        mv = work.tile([128, 2], F32, tag="bn_mv")
        nc.vector.bn_aggr(mv[:rows_n], st[:rows_n])
        sd = work.tile([128, 1], F32, tag="bn_sd")
        nc.scalar.activation(sd[:rows_n], mv[:rows_n, 1:2], AF.Sqrt,
                             bias=eps_t[:rows_n])
        nc.vector.reciprocal(sd[:rows_n], sd[:rows_n])
        if g_b is None:  # final LN: gain/bias folded into W_proj on host
            nc.vector.tensor_scalar(out_ap, x_ap, mv[:rows_n, 0:1],
                                    sd[:rows_n], SUB, MULT)
            return
        t = work.tile([128, NPOS], F32, tag="ln_t")
        nc.vector.tensor_scalar(t[:rows_n], x_ap, mv[:rows_n, 0:1], sd[:rows_n],
                                SUB, MULT)
        nc.vector.tensor_mul(t[:rows_n], t[:rows_n], g_b[:rows_n])
        nc.vector.tensor_add(out_ap, t[:rows_n], b_b[:rows_n])

    # ---------------- embedding ----------------
    X_t, XT_t = {}, {}
    for p in range(2):  # position-major X
        ps = psw.tile([128, 512], F32, tag="qk")
        seed_bias(ps[:, :E], rrow("bemb_r"), 128)
        nc.tensor.matmul(ps[:, :E], xT_sb[:, p * 128:(p + 1) * 128],
                         Wemb_sb[:], start=False, stop=True)
        t = const.tile([128, NPOS], F32, tag=f"X{p}")
        nc.vector.tensor_copy(t[:], ps[:, :E])
        X_t[p] = t
    for k in range(2):  # channel-major XT
        ps = psw.tile([128, 512], F32, tag="qk")
        nc.tensor.matmul(ps[:, :NPOS], Wemb_sb[:, k * 128:(k + 1) * 128],
                         xT_sb[:], start=True, stop=True)
        t = const.tile([128, NPOS], WDT, tag=f"XT{k}")
        nc.vector.tensor_scalar_add(t[:], ps[:, :NPOS],
                                    fb("bemb_pp")[:, k:k + 1])
        XT_t[k] = t

    # ---------------- transformer layers ----------------
    for l in range(LYR):
        qlo, qhi = (0, NPOS) if l == 0 else (QLO2, NPOS)
        qw = qhi - qlo
        pos_chunks = ([(0, 0, 128, 0), (1, 0, 128, 128)] if l == 0
                      else [(2, 0, qw, 0)])
        # (X-tile idx, row offset in tile, nrows, query-col offset)

        # ---- K projection -> KT channel-major bf16 [128, 256] x2
        KT = {}
        for Jt in range(2):
            ps = psw.tile([128, 512], F32, tag="qk")
            for k in range(2):
                nc.tensor.matmul(
                    ps[:, :NPOS],
                    bf(("Wk", l, k))[:, Jt * 128:(Jt + 1) * 128],
                    XT_t[k][:], start=(k == 0), stop=(k == 1))
            t = work.tile([128, NPOS], BF16, tag=f"KT{Jt}")
            nc.vector.tensor_scalar_add(t[:], ps[:, :NPOS],
                                        fb(("bk", l))[:, Jt:Jt + 1])
            KT[Jt] = t

        # ---- V projection -> VO[pc] [128, 8, 64] bf16: per key chunk cp,
        # cols [0:32) = V channels, cols [32:64) = ones.  The combined
        # [V | 1] stationary makes ONE M=64 matmul per exp-region produce
        # both the A@V partial and the softmax row-sum.
        VO = {}
        for pc in range(2):
            ps = psw.tile([128, 512], F32, tag="qk")
            seed_bias(ps[:, :E], wrow(("bv", l)), 128)
            for k in range(2):
                nc.tensor.matmul(
                    ps[:, :E], XT_t[k][:, pc * 128:(pc + 1) * 128],
                    bf(("Wv", l, k))[:], start=False, stop=(k == 1))
            t = work.tile([128, 8, 64], BF16, tag=f"VO{pc}")
            nc.vector.tensor_copy(
                t[:, :, 0:32],
                ps[:, :E].rearrange("p (c d) -> p c d", d=32))
            nc.vector.memset(t[:, :, 32:64], 1.0)
            VO[pc] = t

        # ---- Q projection -> Qs_dbl [128, 2, 2, 4, qw] bf16
        # dims [part, h, dup, r, q]; rotation r written to both dups so a
        # row strip i can read 4 DOUBLED slots starting at (4-i)%4, which
        # enumerates q-chunks c = 4h+0..3 in ascending order.
        Qs_dbl = work.tile([128, 2, 2, 4, qw], BF16, tag=f"qsdbl{l}")
        for h in range(2):
            ps = psw.tile([128, 512], F32, tag="qk")
            for k in range(2):
                nc.tensor.matmul(
                    ps[:, :qw],
                    bf(("Wq", l, k))[:, h * 128:(h + 1) * 128],
                    XT_t[k][:, qlo:qhi], start=(k == 0), stop=(k == 1))
            tf = work.tile([128, NPOS], F32, tag="qtmp")
            nc.vector.tensor_scalar_add(tf[:, :qw], ps[:, :qw],
                                        fb(("bq", l))[:, h:h + 1])
            nc.vector.tensor_mul(Qs_dbl[:, h, 0, 0, :],
                                 tf[:, :qw], fb(("D", h))[:, qlo:qhi])
        for r in range(1, 4):
            for h in range(2):
                ps = psw.tile([128, 512], F32, tag="qk")
                nc.tensor.matmul(ps[:, :qw], bf(("Prot", r - 1))[:],
                                 Qs_dbl[:, h, 0, 0, :],
                                 start=True, stop=True)
                nc.vector.tensor_copy(Qs_dbl[:, h, 0, r, :], ps[:, :qw])
        for h in range(2):  # duplicate the 4 slots (wrap-around reads)
            nc.vector.tensor_copy(Qs_dbl[:, h, 1, :, :], Qs_dbl[:, h, 0, :, :])

        # ---- attention: ST -> exp -> [A@V | rowsum] accumulated in PSUM
        # OTR[64h+d,    c*qw+q] = attention out, q-chunk c=4h+c_local
        # OTR[64h+32+d, c*qw+q] = softmax denominator (identical over d)
        # zeroed by memset; all matmuls accumulate with start=False.
        OTR = psacc.tile([128, 1024], F32, tag="otr")
        nc.vector.memset(OTR[:], 0.0)
        qv = Qs_dbl[:].rearrange("p h u r q -> p (h u r) q")  # [128,16,qw]
        nslot = min(4, 512 // qw)  # c-slots per matmul: 2 for L1, 4 for L2
        ng = 4 // nslot            # score banks per (i,h): 2 for L1, 1 L2
        for J in range(2):          # key c'-quad
            for pc in range(2):     # key position chunk
                # QK: consecutive matmuls walk strips i=2a, 2a+1, ... so
                # the K=32 row-tiles stream concurrently in the PE array.
                AT = {}
                for h in range(2):
                    for g in range(ng):
                        for a in range(2):
                            sc = scp.tile([128, 2, 512], F32, tag="sc")
                            at = atp.tile([128, 2, 512], BF16, tag="at")
                            for b in range(2):
                                i = 2 * a + b
                                s0 = (4 - i) % 4
                                nc.tensor.matmul(
                                    sc[:, b, :nslot * qw],
                                    KT[J][32 * i:32 * (i + 1),
                                          pc * 128:(pc + 1) * 128],
                                    qv[32 * i:32 * (i + 1),
                                       8 * h + s0 + g * nslot:
                                       8 * h + s0 + (g + 1) * nslot, :],
                                    start=True, stop=True,
                                    tile_position=(32 * i, 0))
                            nc.scalar.activation(at[:, :, :nslot * qw],
                                                 sc[:, :, :nslot * qw],
                                                 AF.Exp)
                            AT[(h, g, a)] = at
                # AV + rowsum: one M=64 matmul per psum bank of scores
                for i in range(4):  # key chunk cp = 4J+i
                    cp = 4 * J + i
                    for h in range(2):
                        for o in range(ng):
                            nc.tensor.matmul(
                                OTR[64 * h:64 * h + 64,
                                    o * 512:o * 512 + nslot * qw],
                                VO[pc][:, cp, :],
                                AT[(h, o, i // 2)][:, i % 2, :nslot * qw],
                                start=False, stop=False,
                                skip_group_check=True,
                                tile_position=(0, 64 * h))

        # ---- normalize in the [d, (c,q)] layout (no redistribution):
        # copy RS rows down 32 partitions, recip in place, multiply the
        # PSUM accumulator directly -> bf16 otn consumed by WoR matmuls.
        rsal = work.tile([128, 1024], F32, tag="rsal")
        rinv = work.tile([128, 1024], F32, tag="rinv")
        otn = work.tile([128, 1024], WDT, tag="otn")
        for h in range(2):
            nc.vector.tensor_copy(rsal[64 * h:64 * h + 32, :4 * qw],
                                  OTR[64 * h + 32:64 * h + 64, :4 * qw])
            nc.vector.reciprocal_approx_fast(
                rinv[64 * h:64 * h + 32, :4 * qw],
                rsal[64 * h:64 * h + 32, :4 * qw])
            nc.vector.tensor_tensor(otn[64 * h:64 * h + 32, :4 * qw],
                                    OTR[64 * h:64 * h + 32, :4 * qw],
                                    rinv[64 * h:64 * h + 32, :4 * qw], MULT)

        # ---- O @ Wo + bo + residual -> LN1 -> xa
        xa = {}
        for ci, (xi, ro, nr, co) in enumerate(pos_chunks):
            ps = psw.tile([128, 512], F32, tag="qk")
            seed_bias(ps[:nr, :E], wrow(("bo", l)), nr)
            for h in range(2):
                for c in range(4):
                    nc.tensor.matmul(
                        ps[:nr, :E],
                        otn[64 * h:64 * h + 32,
                            c * qw + co:c * qw + co + nr],
                        bf(("WoR", l))[64 * h:64 * h + 32,
                                       c * E:(c + 1) * E],
                        start=False, stop=(h == 1 and c == 3),
                        tile_position=(64 * h, 0))
            res = work.tile([128, NPOS], F32, tag=f"res{ci}")
            nc.vector.tensor_add(res[:nr], ps[:nr, :E],
                                 X_t[xi][ro:ro + nr, :])
            t = work.tile([128, NPOS], F32, tag=f"xa{ci}")
            layernorm(res[:nr], nr, ln_b[("ln1g", l)], ln_b[("ln1b", l)],
                      t[:nr])
            xa[ci] = t

        # ---- transpose xa -> xaT channel-major (copies on the idle ACT)
        xaT = {}
        for k in range(2):
            t = work.tile([128, NPOS], WDT, tag=f"xaT{k}")
            for ci, (_, _, nr, co) in enumerate(pos_chunks):
                ps = psw.tile([128, 512], F32, tag="qk")
                nc.tensor.transpose(ps[:, :nr],
                                    xa[ci][:nr, k * 128:(k + 1) * 128],
                                    ident[:nr, :nr])
                nc.scalar.copy(t[:, co:co + nr], ps[:, :nr])
            xaT[k] = t

        # ---- FFN: H1T = relu(W1.T x + b1) channel-major bf16 [128, qw] x8
        H1T = {}
        for dk in range(8):
            ps = psw.tile([128, 512], F32, tag="qk")
            for k in range(2):
                nc.tensor.matmul(
                    ps[:, :qw],
                    bf(("W1", l, k))[:, dk * 128:(dk + 1) * 128],
                    xaT[k][:, :qw], start=(k == 0), stop=(k == 1))
            t = work.tile([128, NPOS], BF16, tag=f"H1T{dk}")
            nc.vector.tensor_scalar(t[:, :qw], ps[:, :qw],
                                    fb(("b1", l))[:, dk:dk + 1], 0.0,
                                    ADD, MAX)
            H1T[dk] = t

        # ---- FF = relu(H1 @ W2 + b2); X_next = LN2(xa + FF)
        newX = {}
        for ci, (_, _, nr, co) in enumerate(pos_chunks):
            ps = psw.tile([128, 512], F32, tag="qk")
            seed_bias(ps[:nr, :E], wrow(("b2", l)), nr)
            for dk in range(8):
                nc.tensor.matmul(
                    ps[:nr, :E], H1T[dk][:, co:co + nr],
                    bf(("W2", l, dk))[:], start=False, stop=(dk == 7))
            t = work.tile([128, NPOS], F32, tag=f"ff{ci}")
            nc.vector.tensor_scalar_max(t[:nr], ps[:nr, :E], 0.0)
            res2 = work.tile([128, NPOS], F32, tag=f"res2{ci}")
            nc.vector.tensor_add(res2[:nr], t[:nr], xa[ci][:nr])
            xn = const.tile([128, NPOS], F32, tag=f"Xn{l}{ci}")
            layernorm(res2[:nr], nr, ln_b[("ln2g", l)], ln_b[("ln2b", l)],
                      xn[:nr])
            newX[ci] = xn

        if l == 0:
            # DVE can't read >32 partitions at a nonzero base: shift the
            # output positions [160,256) to partition base 0 for the L2
            # residual add.
            x2res = const.tile([128, NPOS], F32, tag="x2res")
            dma(out=x2res[0:NPOS - QLO2, :],
                in_=newX[1][QLO2 - 128:128, :])
            X_t = {0: newX[0], 1: newX[1], 2: x2res}
            XT_t = {}
            for k in range(2):
                t = const.tile([128, NPOS], WDT, tag=f"X1T{k}")
                for ci in range(2):
                    ps = psw.tile([128, 512], F32, tag="qk")
                    nc.tensor.transpose(ps[:, :128],
                                        newX[ci][:, k * 128:(k + 1) * 128],
                                        ident[:])
                    nc.vector.tensor_copy(t[:, ci * 128:(ci + 1) * 128],
                                          ps[:, :128])
                XT_t[k] = t
        else:
            X2 = newX[0]  # [PRED, 256]

    # ---------------- final LN + projection ----------------
    xf = work.tile([128, NPOS], F32, tag="xf")
    layernorm(X2[:PRED], PRED, None, None, xf[:PRED])
    xfT = {}
    for k in range(2):
        ps = psw.tile([128, 512], F32, tag="qk")
        nc.tensor.transpose(ps[:, :PRED], xf[:PRED, k * 128:(k + 1) * 128],
                            ident[:PRED, :PRED])
        t = work.tile([128, 128], F32, tag=f"xfT{k}")
        nc.vector.tensor_copy(t[:, :PRED], ps[:, :PRED])
        xfT[k] = t
    ps = psw.tile([128, 512], F32, tag="qk")
    nc.tensor.matmul(ps[:PRED, 0:1], ones_row[0:1, :PRED], rrow("bproj"),
                     start=True, stop=False)
    for k in range(2):
        nc.tensor.matmul(ps[:PRED, 0:1], xfT[k][:, :PRED],
                         fb("Wp2")[:, k:k + 1],
                         start=False, stop=(k == 1))
    ot = work.tile([128, 1], F32, tag="outsb")
    nc.vector.tensor_copy(ot[:PRED], ps[:PRED, 0:1])
    nc.sync.dma_start(out=outs["out"][:], in_=ot[:PRED, :])


# ======================= host side =======================

def _rot_matrices():
    """P_r[k, m] = 1 iff k = 32*((m//32 + r) % 4) + m % 32, r = 1..3."""
    mats = np.zeros((3, 128, 128), np.float32)
    for r in range(1, 4):
        for m in range(128):
            mats[r - 1, 32 * ((m // 32 + r) % 4) + m % 32, m] = 1.0
    return mats


def _make_in_maps(inputs):
    import ml_dtypes
    f = np.float32
    bh = ml_dtypes.bfloat16
    x_enc = np.asarray(inputs["x_enc"], f)
    td = np.asarray(inputs["time_diffs"], f)

    blob = np.zeros((128, BF_COLS), bh)
    rot = _rot_matrices()
    for r in range(3):
        blob[:, BF_OFF[("Prot", r)]:BF_OFF[("Prot", r)] + 128] = rot[r]
    for l in range(LYR):
        for nm in ("Wq", "Wk", "Wv"):
            w = np.asarray(inputs[nm], f)[l]
            for k in range(2):
                blob[:, BF_OFF[(nm, l, k)]:BF_OFF[(nm, l, k)] + E] = \
                    w[k * 128:(k + 1) * 128, :]
        wo = np.asarray(inputs["Wo"], f)[l]
        wor = np.zeros((128, 4 * E), f)
        for h in range(2):
            for c in range(4):
                wor[64 * h:64 * h + 32, c * E:(c + 1) * E] = \
                    wo[32 * (4 * h + c):32 * (4 * h + c) + 32, :]
        blob[:, BF_OFF[("WoR", l)]:BF_OFF[("WoR", l)] + 4 * E] = wor
        w1 = np.asarray(inputs["W1"], f)[l]
        for k in range(2):
            blob[:, BF_OFF[("W1", l, k)]:BF_OFF[("W1", l, k)] + DFF] = \
                w1[k * 128:(k + 1) * 128, :]
        w2 = np.asarray(inputs["W2"], f)[l]
        for dk in range(8):
            blob[:, BF_OFF[("W2", l, dk)]:BF_OFF[("W2", l, dk)] + E] = \
                w2[dk * 128:(dk + 1) * 128, :]

    fblob_base = np.zeros((128, F_COLS), f)
    for l in range(LYR):
        for nm, w in (("bq", 2), ("bk", 2), ("b1", 8)):
            arr = np.asarray(inputs[nm], f)[l].reshape(w, 128).T
            fblob_base[:, F_OFF[(nm, l)]:F_OFF[(nm, l)] + w] = arr
    fblob_base[:, F_OFF["bemb_pp"]:F_OFF["bemb_pp"] + 2] = \
        np.asarray(inputs["b_emb"], f).reshape(2, 128).T
    fblob_base[:, F_OFF["Wp2"]:F_OFF["Wp2"] + 2] = \
        (np.asarray(inputs["lnf_g"], f)
         * np.asarray(inputs["W_proj"], f)[:, 0]).reshape(2, 128).T

    rows = np.zeros((1, R_COLS), f)
    for nm, src in (("ln1g", "ln1_g"), ("ln1b", "ln1_b"),
                    ("ln2g", "ln2_g"), ("ln2b", "ln2_b")):
        for l in range(LYR):
            rows[0, R_OFF[(nm, l)]:R_OFF[(nm, l)] + E] = \
                np.asarray(inputs[src], f)[l]
    rows[0, R_OFF["bemb_r"]:R_OFF["bemb_r"] + E] = \
        np.asarray(inputs["b_emb"], f)
    # final LN gain/bias folded into the projection column:
    # (z*g + b) @ Wp0 + bp0 = z @ (g*Wp0) + (b@Wp0 + bp0)
    wp0 = np.asarray(inputs["W_proj"], f)[:, 0]
    lnfg = np.asarray(inputs["lnf_g"], f)
    lnfb = np.asarray(inputs["lnf_b"], f)
    rows[0, R_OFF["bproj"]] = (np.asarray(inputs["b_proj"], f)[0]
                               + float(lnfb @ wp0))

    wrows = np.zeros((1, WR_COLS), bh)
    for nm, src in (("bv", "bv"), ("bo", "bo"), ("b2", "b2")):
        for l in range(LYR):
            wrows[0, WR_OFF[(nm, l)]:WR_OFF[(nm, l)] + E] = \
                np.asarray(inputs[src], f)[l]

    maps = []
    for b in range(B):
        fblob = fblob_base.copy()
        # decay tiles: D[h][32j+d, q] = SCALE * exp(-td[q*8 + 4h+j] / F)
        dec = SCALE * np.exp(-td[b].reshape(NPOS, 8) / FACTOR)  # [q, chunk]
        for h in range(2):
            tile_ = np.repeat(dec[:, 4 * h:4 * h + 4].T, 32, axis=0)
            fblob[:, F_OFF[("D", h)]:F_OFF[("D", h)] + NPOS] = tile_
        xwa = np.zeros((D, 2 * NPOS), f)
        xwa[:, :NPOS] = x_enc[b, P0:P0 + NPOS, :].T
        xwa[:, NPOS:] = np.asarray(inputs["W_emb"], f)
        maps.append({
            "blob": blob, "fblob": np.ascontiguousarray(fblob),
            "rows": rows, "wrows": wrows,
            "xw": np.ascontiguousarray(xwa),
        })
    return maps


def _run(in_maps, check_with_sim=False, check_with_hw=True, **kw):
    from concourse.bass_test_utils import run_kernel

    n = len(in_maps)
    out_like = {"out": np.zeros((PRED, 1), np.float32)}
    res = run_kernel(
        lambda tc, outs, ins: chaos_kernel(tc, outs, ins),
        None,
        in_maps if n > 1 else in_maps[0],
        output_like=[out_like] * n if n > 1 else out_like,
        bass_type=tile.TileContext,
        num_cores=n,
        check_with_sim=check_with_sim,
        check_with_hw=check_with_hw,
        trace_sim=False,
        **kw,
    )
    return res


def kernel(**inputs):
    in_maps = _make_in_maps(inputs)
    res = _run(in_maps)
    out = np.stack(
        [list(res.results[b].values())[0].reshape(PRED) for b in range(B)])
    return out.astype(np.float32)
